# revision 1
# baseline (speedup 1.0000x reference)
"""GATNE model (attention over edge types + ragged segment-mean + FC) on 8
Trainium2 NeuronCores via Bass/Tile.

Strategy (self-contained; hardcoded for the spec shapes):
  Phase 1 (SPMD, data-parallel over B): samples are sorted by train_type on
  the host so every 128-sample tile is type-homogeneous, padded, and dealt to
  the 8 cores. Per tile: neighbor gather-sum (SWDGE dma_gather from the
  t-major node_type_embeddings table), attention (PE matmuls + ACT
  tanh/exp), per-sample trans_weights matmul, row L2-normalize. Output: the
  normalized embedding table shard per core.
  Host relays the 8 shards into one table (no on-device collective needed).
  Phase 2 (SPMD, sharded by segment blocks of 256): dma_gather of the
  entries' embedding rows, segment-sum via one-hot matmuls accumulated in
  PSUM, mean+normalize, FC (PE) and final l2norm.
"""
import contextlib
import sys

sys.path.insert(0, "/opt/trn_rl_repo")

import numpy as np

import concourse.bacc as bacc
import concourse.bass as bass
import concourse.mybir as mybir
import concourse.tile as tile
from concourse.bass_utils import run_bass_kernel_spmd
from concourse.masks import make_identity

F32 = mybir.dt.float32
F32R = mybir.dt.float32r
BF16 = mybir.dt.bfloat16
I16 = mybir.dt.int16
AF = mybir.ActivationFunctionType
ALU = mybir.AluOpType

N_CORES = 8
NUM_NODES = 2500
T = 4
EMB = 256
U = 64
DIM_A = 32
EMBED_SIZE = 512
NEIGH = 10
B = NUM_NODES * T
NUM_SMS = 64
MAX_REGION = 32
NSEG = NUM_SMS * MAX_REGION  # 2048
SEG_PER_CORE = NSEG // N_CORES  # 256
E_TOTAL = 131072
P = 128


def _wrap16(flat):
    """dma_gather index layout: idx i -> partition i%16, column i//16,
    replicated across the 8 partition groups (one per Q7 core)."""
    n = flat.shape[0]
    assert n % 16 == 0
    a = flat.reshape(n // 16, 16).T.astype(np.int16)  # [16, n//16]
    return np.tile(a, (8, 1))  # [128, n//16]


# ---------------------------------------------------------------------------
# walrus post-pass: CoreV3 codegen rejects >1 sem wait on a TPB_CTRL (Drain);
# split the excess onto injected wait-only drains placed just before.
# ---------------------------------------------------------------------------
def _split_drain_waits(nc, max_waits=1):
    for bb in nc.main_func.blocks:
        out = []
        for ins in bb.instructions:
            si = ins.sync_info
            if (
                type(ins).__name__ == "InstDrain"
                and si is not None
                and si.on_wait is not None
                and len(si.on_wait) > max_waits
            ):
                waits = list(si.on_wait)
                extra, keep = waits[:-max_waits], waits[-max_waits:]
                for i in range(0, len(extra), max_waits):
                    d = mybir.InstDrain(
                        name=nc.get_next_instruction_name(),
                        ins=[],
                        outs=[],
                        bass_is_fusable=False,
                    )
                    d.engine = ins.engine
                    d.sync_info = mybir.SyncInfo(
                        on_wait=extra[i : i + max_waits], on_update=[]
                    )
                    out.append(d)
                ins.sync_info = mybir.SyncInfo(
                    on_wait=keep, on_update=list(si.on_update or [])
                )
            out.append(ins)
        bb.instructions = out


# ---------------------------------------------------------------------------
# Phase-1 program builder
# ---------------------------------------------------------------------------
def build_phase1(TPC, reps=1):
    nc = bacc.Bacc("TRN2", debug=False)
    ttab = nc.dram_tensor("ttab", [T * NUM_NODES, U], F32, kind="ExternalInput")
    ntab = nc.dram_tensor("ntab", [NUM_NODES, EMB], F32, kind="ExternalInput")
    nbridx = nc.dram_tensor("nbridx", [P, TPC * 320], I16, kind="ExternalInput")
    neidx = nc.dram_tensor("neidx", [P, TPC * 8], I16, kind="ExternalInput")
    s1w = nc.dram_tensor("s1w", [U, TPC * DIM_A], F32, kind="ExternalInput")
    s2w = nc.dram_tensor("s2w", [DIM_A, TPC], F32, kind="ExternalInput")
    ww = nc.dram_tensor("ww", [U, TPC * EMB], F32, kind="ExternalInput")
    lne_out = nc.dram_tensor("lne", [P, TPC * EMB], F32, kind="ExternalOutput")

    with tile.TileContext(nc) as tc:
        with (
            tc.tile_pool(name="const", bufs=1) as cpool,
            tc.tile_pool(name="gat", bufs=3) as gpool,
            tc.tile_pool(name="work", bufs=2) as wpool,
            tc.tile_pool(name="ps_tp", bufs=2, space="PSUM") as ps_tp,
            tc.tile_pool(name="ps_h", bufs=2, space="PSUM") as ps_h,
            tc.tile_pool(name="ps_lg", bufs=1, space="PSUM") as ps_lg,
            tc.tile_pool(name="ps_at", bufs=1, space="PSUM") as ps_at,
            tc.tile_pool(name="ps_dl", bufs=2, space="PSUM") as ps_dl,
        ):
            ident = cpool.tile([P, P], F32)
            make_identity(nc, ident[:])
            s1_sb = cpool.tile([U, TPC * DIM_A], F32)
            nc.sync.dma_start(out=s1_sb[:], in_=s1w[:])
            s2_sb = cpool.tile([DIM_A, TPC], F32)
            nc.sync.dma_start(out=s2_sb[:], in_=s2w[:])
            w_sb = cpool.tile([U, TPC * EMB], F32)
            nc.sync.dma_start(out=w_sb[:], in_=ww[:])
            nbr_sb = cpool.tile([P, TPC * 320], I16)
            nc.sync.dma_start(out=nbr_sb[:], in_=nbridx[:])
            ne_idx_sb = cpool.tile([P, TPC * 8], I16)
            nc.sync.dma_start(out=ne_idx_sb[:], in_=neidx[:])
            lne_sb = cpool.tile([P, TPC * EMB], F32)

            with tc.For_i(0, reps, 1) if reps > 1 else contextlib.nullcontext():
                # full-core node_embeddings gather: [128, TPC, 256]
                ne_g = cpool.tile([P, TPC, EMB], F32, tag="ne_g")
                nc.gpsimd.dma_gather(
                    ne_g[:], ntab[:], ne_idx_sb[:], TPC * P, TPC * P, EMB,
                    single_packet=False,
                )
                for j in range(TPC):
                    # ---- neighbor gather: [128, 40, 64]; free = (n, t, u)
                    g = gpool.tile([P, NEIGH * T, U], F32, tag="g")
                    nc.gpsimd.dma_gather(
                        g[:],
                        ttab[:],
                        nbr_sb[:, j * 320 : (j + 1) * 320],
                        5120,
                        5120,
                        U,
                        single_packet=False,
                    )
                    # ---- sum over 10 neighbors -> nte [128, (t,u)=256]
                    # single strided reduce: view free dim as (tu, n), n inner
                    gr = g[:].rearrange("p (n c) u -> p (c u) n", n=NEIGH)
                    nte = wpool.tile([P, T * U], F32, tag="nte")
                    nc.vector.tensor_reduce(
                        out=nte[:], in_=gr, axis=mybir.AxisListType.X, op=ALU.add
                    )
                    # ---- transpose nte -> per-type [64u, 128b] blocks (base
                    # partition 0 so they can feed matmul rhs)
                    pt = ps_tp.tile([U, T * P], F32, tag="pt")
                    for t in range(T):
                        nc.tensor.transpose(
                            pt[:, t * P : (t + 1) * P],
                            nte[:, t * U : (t + 1) * U],
                            ident[:],
                        )
                    ntet = wpool.tile([U, T * P], F32, tag="ntet")
                    nc.vector.tensor_copy(out=ntet[:], in_=pt[:])

                    # ---- h = tanh(nte @ S1) : 4 matmuls -> psum [32a, (t,b)]
                    h_ps = ps_h.tile([DIM_A, T * P], F32, tag="h_ps")
                    for t in range(T):
                        nc.tensor.matmul(
                            out=h_ps[:, t * P : (t + 1) * P],
                            lhsT=s1_sb[:, j * DIM_A : (j + 1) * DIM_A],
                            rhs=ntet[:, t * P : (t + 1) * P],
                            start=True,
                            stop=True,
                        )
                    h = wpool.tile([DIM_A, T * P], F32, tag="h")
                    nc.scalar.activation(h[:], h_ps[:], AF.Tanh)

                    # ---- logits [128b, 4t]: per t, h_t.T @ s2 (N=1)
                    lg_ps = ps_lg.tile([P, T], F32, tag="lg_ps")
                    for t in range(T):
                        nc.tensor.matmul(
                            out=lg_ps[:, t : t + 1],
                            lhsT=h[:, t * P : (t + 1) * P],
                            rhs=s2_sb[:, j : j + 1],
                            start=True,
                            stop=True,
                        )
                    # ---- softmax over the 4 type slots
                    nmax = wpool.tile([P, 1], F32, tag="nmax")
                    nc.vector.tensor_reduce(
                        out=nmax[:],
                        in_=lg_ps[:],
                        axis=mybir.AxisListType.X,
                        op=ALU.max,
                        negate=True,
                    )
                    ex = wpool.tile([P, T], F32, tag="ex")
                    nc.scalar.activation(ex[:], lg_ps[:], AF.Exp, bias=nmax[:, 0:1])
                    ssum = wpool.tile([P, 1], F32, tag="ssum")
                    nc.vector.tensor_reduce(
                        out=ssum[:], in_=ex[:], axis=mybir.AxisListType.X, op=ALU.add
                    )
                    rs = wpool.tile([P, 1], F32, tag="rs")
                    nc.vector.reciprocal(rs[:], ssum[:])
                    att = wpool.tile([P, T], F32, tag="att")
                    nc.vector.tensor_scalar_mul(att[:], ex[:], rs[:, 0:1])

                    # ---- agg[b,u] = sum_t att[b,t] * nte[b,t,u]
                    ntev = nte[:].rearrange("p (t u) -> p t u", t=T)
                    attb = att[:, :, None].to_broadcast([P, T, U])
                    tmp = wpool.tile([P, T, U], F32, tag="tmp")
                    nc.vector.tensor_tensor(
                        out=tmp[:], in0=ntev, in1=attb, op=ALU.mult
                    )
                    agg = wpool.tile([P, U], F32, tag="agg")
                    tmpv = tmp[:].rearrange("p t u -> p u t")
                    nc.vector.tensor_reduce(
                        out=agg[:], in_=tmpv, axis=mybir.AxisListType.X, op=ALU.add
                    )
                    # ---- aggT [64, 128]
                    at_ps = ps_at.tile([U, P], F32, tag="at_ps")
                    nc.tensor.transpose(at_ps[:], agg[:], ident[:])
                    aggT = wpool.tile([U, P], F32, tag="aggT")
                    nc.vector.tensor_copy(out=aggT[:], in_=at_ps[:])

                    # ---- delta = aggT.T @ W  -> [128b, 256e]
                    dl_ps = ps_dl.tile([P, EMB], F32, tag="dl_ps")
                    nc.tensor.matmul(
                        out=dl_ps[:],
                        lhsT=aggT[:],
                        rhs=w_sb[:, j * EMB : (j + 1) * EMB],
                        start=True,
                        stop=True,
                    )
                    # ---- ne_new = node_embed + delta; L2 normalize rows
                    nen = wpool.tile([P, EMB], F32, tag="nen")
                    nc.vector.tensor_add(out=nen[:], in0=ne_g[:, j, :], in1=dl_ps[:])
                    sq = wpool.tile([P, EMB], F32, tag="sq")
                    ss = wpool.tile([P, 1], F32, tag="ss")
                    nc.scalar.activation(sq[:], nen[:], AF.Square, accum_out=ss[:])
                    nrm = wpool.tile([P, 1], F32, tag="nrm")
                    nc.scalar.activation(nrm[:], ss[:], AF.Sqrt)
                    nc.vector.tensor_scalar_max(nrm[:], nrm[:], 1e-12)
                    inv = wpool.tile([P, 1], F32, tag="inv")
                    nc.vector.reciprocal(inv[:], nrm[:])
                    nc.vector.tensor_scalar_mul(
                        lne_sb[:, j * EMB : (j + 1) * EMB], nen[:], inv[:, 0:1]
                    )
            nc.sync.dma_start(out=lne_out[:], in_=lne_sb[:])

    nc.compile()
    _split_drain_waits(nc)
    return nc


# ---------------------------------------------------------------------------
# Phase-2 program builder
# ---------------------------------------------------------------------------
def build_phase2(EPT_C, lne_rows, reps=1):
    """EPT_C: entry tiles per 128-segment chunk (2 chunks per core), uniform
    across all 16 (core, chunk) pairs. lne_rows: rows of the relayed
    embedding table. Each entry tile touches only its chunk's 128 segments,
    so the segment-sum needs a single one-hot matmul per tile."""
    GCALL = 16  # entry tiles per dma_gather call
    EPT = 2 * EPT_C
    nc = bacc.Bacc("TRN2", debug=False)
    lne = nc.dram_tensor("lne", [lne_rows, EMB], F32, kind="ExternalInput")
    eidx = nc.dram_tensor("eidx", [P, EPT * 8], I16, kind="ExternalInput")
    lid = nc.dram_tensor("lid", [P, EPT], F32, kind="ExternalInput")
    iota = nc.dram_tensor("iota", [P, P], F32, kind="ExternalInput")
    recip = nc.dram_tensor("recip", [P, 2], F32, kind="ExternalInput")
    fcwt = nc.dram_tensor("fcwt", [EMB, EMBED_SIZE], F32, kind="ExternalInput")
    fcb = nc.dram_tensor("fcb", [P, EMBED_SIZE], F32, kind="ExternalInput")
    out = nc.dram_tensor("out", [SEG_PER_CORE, EMBED_SIZE], F32, kind="ExternalOutput")

    with tile.TileContext(nc) as tc:
        with (
            tc.tile_pool(name="const", bufs=1) as cpool,
            tc.tile_pool(name="gat", bufs=4) as gpool,
            tc.tile_pool(name="oh", bufs=8) as ohpool,
            tc.tile_pool(name="work", bufs=2) as wpool,
            tc.tile_pool(name="ps_acc", bufs=2, space="PSUM") as ps_acc,
            tc.tile_pool(name="ps_tp", bufs=2, space="PSUM") as ps_tp,
            tc.tile_pool(name="ps_fc", bufs=2, space="PSUM") as ps_fc,
        ):
            ident = cpool.tile([P, P], F32)
            make_identity(nc, ident[:])
            iota_sb = cpool.tile([P, P], F32)
            nc.sync.dma_start(out=iota_sb[:], in_=iota[:])
            recip_sb = cpool.tile([P, 2], F32)
            nc.sync.dma_start(out=recip_sb[:], in_=recip[:])
            fcwt0 = cpool.tile([P, EMBED_SIZE], F32, tag="fcwt0")
            fcwt1 = cpool.tile([P, EMBED_SIZE], F32, tag="fcwt1")
            fcwt_sb = [fcwt0, fcwt1]
            for i in range(2):
                nc.sync.dma_start(out=fcwt_sb[i][:], in_=fcwt[i * P : (i + 1) * P, :])
            fcb_sb = cpool.tile([P, EMBED_SIZE], F32)
            nc.sync.dma_start(out=fcb_sb[:], in_=fcb[:])
            eidx_sb = cpool.tile([P, EPT * 8], I16)
            nc.sync.dma_start(out=eidx_sb[:], in_=eidx[:])
            lid_sb = cpool.tile([P, EPT], F32)
            nc.sync.dma_start(out=lid_sb[:], in_=lid[:])
            eps8 = cpool.tile([P, 1], F32)
            nc.vector.memset(eps8[:], 1e-8)

            with tc.For_i(0, reps, 1) if reps > 1 else contextlib.nullcontext():
                acc0 = ps_acc.tile([P, EMB], F32, tag="acc0")
                acc1 = ps_acc.tile([P, EMB], F32, tag="acc1")
                acc = [acc0, acc1]
                nc.vector.memset(acc[0][:], 0.0)
                nc.vector.memset(acc[1][:], 0.0)
                for ch in range(2):
                    done = 0
                    while done < EPT_C:
                        nt = min(GCALL, EPT_C - done)
                        j0 = ch * EPT_C + done
                        g = gpool.tile([P, GCALL, EMB], F32, tag="g")
                        nc.gpsimd.dma_gather(
                            g[:, :nt, :],
                            lne[:],
                            eidx_sb[:, j0 * 8 : (j0 + nt) * 8],
                            nt * P,
                            nt * P,
                            EMB,
                            single_packet=False,
                        )
                        for jj in range(nt):
                            j = j0 + jj
                            oh = ohpool.tile([P, P], F32, tag="oh")
                            nc.vector.tensor_scalar(
                                out=oh[:],
                                in0=iota_sb[:],
                                scalar1=lid_sb[:, j : j + 1],
                                scalar2=None,
                                op0=ALU.is_equal,
                            )
                            nc.tensor.matmul(
                                out=acc[ch][:],
                                lhsT=oh[:],
                                rhs=g[:, jj, :],
                                start=False,
                                stop=False,
                                skip_group_check=True,
                            )
                        done += nt
                # ---- mean + normalize -> smn [2][128, 256]
                smn = []
                for half in range(2):
                    mean = wpool.tile([P, EMB], F32, tag=f"mean{half}")
                    nc.vector.tensor_scalar_mul(
                        mean[:], acc[half][:], recip_sb[:, half : half + 1]
                    )
                    sq = wpool.tile([P, EMB], F32, tag="p2sq")
                    ss = wpool.tile([P, 1], F32, tag="p2ss")
                    nc.scalar.activation(sq[:], mean[:], AF.Square, accum_out=ss[:])
                    nrm = wpool.tile([P, 1], F32, tag="p2nrm")
                    nc.scalar.activation(nrm[:], ss[:], AF.Sqrt)
                    nc.vector.tensor_scalar_max(nrm[:], nrm[:], 1e-12)
                    inv = wpool.tile([P, 1], F32, tag="p2inv")
                    nc.vector.reciprocal(inv[:], nrm[:])
                    sm = wpool.tile([P, EMB], F32, tag=f"smn{half}")
                    nc.vector.tensor_scalar_mul(sm[:], mean[:], inv[:, 0:1])
                    smn.append(sm)
                # ---- transpose smn -> smnT [emb-half][128, 256(seg)]
                smnT = []
                for eh in range(2):
                    tp = ps_tp.tile([P, 2 * P], F32, tag="tp")
                    for half in range(2):
                        nc.tensor.transpose(
                            tp[:, half * P : (half + 1) * P],
                            smn[half][:, eh * P : (eh + 1) * P],
                            ident[:],
                        )
                    st = wpool.tile([P, 2 * P], F32, tag=f"smnT{eh}")
                    nc.vector.tensor_copy(out=st[:], in_=tp[:])
                    smnT.append(st)
                # ---- FC + bias + l2norm
                for m in range(2):
                    fc_ps = ps_fc.tile([P, EMBED_SIZE], F32, tag="fc_ps")
                    for kh in range(2):
                        nc.tensor.matmul(
                            out=fc_ps[:],
                            lhsT=smnT[kh][:, m * P : (m + 1) * P],
                            rhs=fcwt_sb[kh][:],
                            start=(kh == 0),
                            stop=(kh == 1),
                        )
                    xx = wpool.tile([P, EMBED_SIZE], F32, tag="xx")
                    nc.vector.tensor_add(out=xx[:], in0=fc_ps[:], in1=fcb_sb[:])
                    sq = wpool.tile([P, EMBED_SIZE], F32, tag="p3sq")
                    ss = wpool.tile([P, 1], F32, tag="p3ss")
                    nc.scalar.activation(sq[:], xx[:], AF.Square, accum_out=ss[:])
                    nrm = wpool.tile([P, 1], F32, tag="p3nrm")
                    # n = sqrt(ss + 1e-8) + 1e-8
                    nc.scalar.activation(nrm[:], ss[:], AF.Sqrt, bias=eps8[:, 0:1])
                    nc.vector.tensor_scalar_add(nrm[:], nrm[:], 1e-8)
                    inv = wpool.tile([P, 1], F32, tag="p3inv")
                    nc.vector.reciprocal(inv[:], nrm[:])
                    res = wpool.tile([P, EMBED_SIZE], F32, tag="res")
                    nc.vector.tensor_scalar_mul(res[:], xx[:], inv[:, 0:1])
                    nc.sync.dma_start(
                        out=out[m * P : (m + 1) * P, :], in_=res[:]
                    )

    nc.compile()
    _split_drain_waits(nc)
    return nc


# ---------------------------------------------------------------------------
# Host-side orchestration
# ---------------------------------------------------------------------------
def _phase1_prep(train_inputs, train_types, node_neigh):
    order = np.argsort(train_types, kind="stable")
    ts = train_types[order]
    tiles_s, tiles_t = [], []
    for t in range(T):
        idx_t = order[ts == t]
        if len(idx_t) == 0:
            continue
        n_tiles = -(-len(idx_t) // P)
        padded = np.concatenate(
            [idx_t, np.repeat(idx_t[-1:], n_tiles * P - len(idx_t))]
        )
        for jj in range(n_tiles):
            tiles_s.append(padded[jj * P : (jj + 1) * P])
            tiles_t.append(t)
    while len(tiles_s) % N_CORES:
        tiles_s.append(tiles_s[-1])
        tiles_t.append(tiles_t[-1])
    sample_mat = np.stack(tiles_s)  # [TT, 128]
    tile_type = np.asarray(tiles_t)
    TT = sample_mat.shape[0]
    TPC = TT // N_CORES

    flat = sample_mat.reshape(-1)
    slot_of_sample = np.zeros(B, np.int64)
    slot_of_sample[flat[::-1]] = np.arange(TT * P)[::-1]
    return sample_mat, tile_type, TPC, slot_of_sample


def _phase1_inmaps(inputs, sample_mat, tile_type, TPC):
    node_embeddings = np.asarray(inputs["node_embeddings"], np.float32)
    node_type_embeddings = np.asarray(inputs["node_type_embeddings"], np.float32)
    trans_weights = np.asarray(inputs["trans_weights"], np.float32)
    trans_weights_s1 = np.asarray(inputs["trans_weights_s1"], np.float32)
    trans_weights_s2 = np.asarray(inputs["trans_weights_s2"], np.float32)
    train_inputs = np.asarray(inputs["train_inputs"])
    node_neigh = np.asarray(inputs["node_neigh"])

    ttab = np.ascontiguousarray(
        node_type_embeddings.transpose(1, 0, 2).reshape(T * NUM_NODES, U)
    )
    in_maps = []
    for k in range(N_CORES):
        smp = sample_mat[k * TPC : (k + 1) * TPC]  # [TPC, 128]
        ct = tile_type[k * TPC : (k + 1) * TPC]  # [TPC]
        ne_flat = train_inputs[smp].reshape(-1)  # order: tile-major, then p
        ne_idx = _wrap16(ne_flat)
        nn_t = node_neigh[smp]  # [TPC, 128, 4, 10]
        nnb = nn_t + (np.arange(T) * NUM_NODES)[None, None, :, None]
        nnb2 = nnb.transpose(0, 3, 2, 1)  # [TPC, 10, 4, 128]
        nbr_idx = np.concatenate(
            [_wrap16(nnb2[j].reshape(-1)) for j in range(TPC)], axis=1
        )
        s1_all = np.ascontiguousarray(
            trans_weights_s1[ct].transpose(1, 0, 2).reshape(U, TPC * DIM_A)
        )
        w_all = np.ascontiguousarray(
            trans_weights[ct].transpose(1, 0, 2).reshape(U, TPC * EMB)
        )
        s2_blk = np.ascontiguousarray(trans_weights_s2[ct][:, :, 0].T)  # [32, TPC]
        in_maps.append(
            {
                "ttab": ttab,
                "ntab": node_embeddings,
                "nbridx": nbr_idx,
                "neidx": ne_idx,
                "s1w": s1_all,
                "s2w": s2_blk,
                "ww": w_all,
            }
        )
    return in_maps


def _phase2_prep(region_index, region_segment_ids, slot_of_sample, lne_rows_real):
    """Split entries into 16 chunks of 128 segments (2 per core); every chunk
    is padded to the same tile count EPT_C so one SPMD program fits all."""
    seg_ids = np.asarray(region_segment_ids).astype(np.int64)
    new_idx = slot_of_sample[np.asarray(region_index).astype(np.int64)]
    zero_row = lne_rows_real  # index of the all-zero pad row
    bounds = np.searchsorted(seg_ids, np.arange(0, NSEG + 1, P))  # 16 chunks
    n_c = bounds[1:] - bounds[:-1]
    EPT_C = int(max(1, -(-int(n_c.max()) // P)))

    cnt = np.bincount(seg_ids, minlength=NSEG).astype(np.float32)
    recip_all = np.where(cnt > 0, 1.0 / np.maximum(cnt, 1.0), 0.0).astype(np.float32)

    eidx_l, lid_l, recip_l = [], [], []
    for k in range(N_CORES):
        idx_parts, lid_parts = [], []
        for h in range(2):
            c = k * 2 + h
            lo, hi = bounds[c], bounds[c + 1]
            idx_c = new_idx[lo:hi]
            lid_c = (seg_ids[lo:hi] - c * P).astype(np.float32)
            pad = EPT_C * P - (hi - lo)
            pad_lid = lid_c[-1] if len(lid_c) else 0.0
            idx_parts.append(
                np.concatenate([idx_c, np.full(pad, zero_row, np.int64)])
            )
            lid_parts.append(
                np.concatenate([lid_c, np.full(pad, pad_lid, np.float32)])
            )
        idx_k = np.concatenate(idx_parts)
        lid_k = np.concatenate(lid_parts)
        EPT = 2 * EPT_C
        eidx_l.append(_wrap16(idx_k))
        # lid layout: [128, EPT], entry j*128+p -> [p, j]
        lid_l.append(np.ascontiguousarray(lid_k.reshape(EPT, P).T))
        rc = recip_all[k * SEG_PER_CORE : (k + 1) * SEG_PER_CORE]
        recip_l.append(np.ascontiguousarray(rc.reshape(2, P).T))
    return EPT_C, eidx_l, lid_l, recip_l


def _phase2_inmaps(inputs, lne_full, eidx_l, lid_l, recip_l):
    fc_w = np.asarray(inputs["fc_w"], np.float32)
    fc_b = np.asarray(inputs["fc_b"], np.float32)
    fcwt = np.ascontiguousarray(fc_w.T)  # [256, 512]
    fcb = np.broadcast_to(fc_b[None, :], (P, EMBED_SIZE)).copy()
    iota = np.broadcast_to(
        np.arange(P, dtype=np.float32)[None, :], (P, P)
    ).copy()
    in_maps = []
    for k in range(N_CORES):
        in_maps.append(
            {
                "lne": lne_full,
                "eidx": eidx_l[k],
                "lid": lid_l[k],
                "iota": iota,
                "recip": recip_l[k],
                "fcwt": fcwt,
                "fcb": fcb,
            }
        )
    return in_maps



def _run_spmd_retry(nc, in_maps, retries=3, delay=45.0):
    """The axon-tunneled device occasionally reports a transient
    UNAVAILABLE/unrecoverable state; back off and retry."""
    import time as _time

    last = None
    for attempt in range(retries):
        try:
            return run_bass_kernel_spmd(nc, in_maps, list(range(N_CORES)))
        except Exception as e:  # jax.errors.JaxRuntimeError and friends
            last = e
            if attempt + 1 < retries:
                _time.sleep(delay)
    raise last


_P1_CACHE = {}
_P2_CACHE = {}


def kernel(**inputs) -> np.ndarray:
    train_inputs = np.asarray(inputs["train_inputs"])
    train_types = np.asarray(inputs["train_types"])
    node_neigh = np.asarray(inputs["node_neigh"])
    num_sms = int(inputs["num_sms"])
    max_region = int(inputs["max_region"])

    sample_mat, tile_type, TPC, slot_of_sample = _phase1_prep(
        train_inputs, train_types, node_neigh
    )
    TT = sample_mat.shape[0]

    if TPC not in _P1_CACHE:
        _P1_CACHE[TPC] = build_phase1(TPC)
    nc1 = _P1_CACHE[TPC]
    in_maps1 = _phase1_inmaps(inputs, sample_mat, tile_type, TPC)
    res1 = _run_spmd_retry(nc1, in_maps1).results

    # relay: lne rows at global_row = tile*128 + p
    lne_rows_real = TT * P
    lne_full = np.empty((lne_rows_real + P, EMB), np.float32)
    for k in range(N_CORES):
        shard = res1[k]["lne"].reshape(P, TPC, EMB).transpose(1, 0, 2)
        lne_full[k * TPC * P : (k + 1) * TPC * P] = shard.reshape(TPC * P, EMB)
    lne_full[lne_rows_real:] = 0.0

    EPT_C, eidx_l, lid_l, recip_l = _phase2_prep(
        inputs["region_index"], inputs["region_segment_ids"], slot_of_sample,
        lne_rows_real,
    )
    key = (EPT_C, lne_full.shape[0])
    if key not in _P2_CACHE:
        _P2_CACHE[key] = build_phase2(EPT_C, lne_full.shape[0])
    nc2 = _P2_CACHE[key]
    in_maps2 = _phase2_inmaps(inputs, lne_full, eidx_l, lid_l, recip_l)
    res2 = _run_spmd_retry(nc2, in_maps2).results

    out = np.concatenate([res2[k]["out"] for k in range(N_CORES)], axis=0)
    return out.reshape(num_sms, max_region, EMBED_SIZE)



# revision 25
# speedup vs baseline: 3.7865x; 3.7865x over previous
"""GATNE model (attention over edge types + ragged segment-mean + FC) on 8
Trainium2 NeuronCores via Bass/Tile.

v2 strategy (self-contained; hardcoded for the spec shapes). The v1 kernel
was descriptor-bound: 52k dma_gather descriptors in phase 1 and 18k in
phase 2 dominate on HW (SWDGE desc-gen on the Q7). v2 eliminates nearly all
gather descriptors by turning both irregular reductions into dense matmuls
against host-built fp8 count matrices (integer index preprocessing only —
all float math stays on device):

  Phase 1 (SPMD, data-parallel over B): samples sorted by train_type into
  128-sample type-homogeneous tiles. The neighbor gather-sum becomes
  ntet[u, (t,s)] = sum_chunks ttab_chunk[node,u]^T @ A_chunk[node, (t,s)]
  with ttab (bf16) SBUF-resident and A (fp8 counts, 0/1/2...) streamed.
  Attention (PE matmuls + ACT tanh/exp), per-sample trans_weights matmul,
  and a batched row-L2-normalize (single Sqrt -> one act-table switch per
  iteration instead of 18). Output: bf16 embedding table shard per core.
  Host relays the 8 shards into one table (free between phases).

  Phase 2 (SPMD, sharded by segment blocks: 256 segments per core): the
  ragged segment-sum becomes seg_acc[seg, e] = C^T[row, seg]^T @ lne[row, e]
  streaming both the bf16 lne table (5.2 MB) and the fp8 count matrix C
  (2.6 MB) — zero gather descriptors. Then mean + normalize + FC + l2norm.
"""
import contextlib
import sys

sys.path.insert(0, "/opt/trn_rl_repo")

import numpy as np
import ml_dtypes

import concourse.bacc as bacc
import concourse.bass as bass
import concourse.mybir as mybir
import concourse.tile as tile
from concourse.bass_utils import run_bass_kernel_spmd
from concourse.masks import make_identity

F32 = mybir.dt.float32
BF16 = mybir.dt.bfloat16
FP8 = mybir.dt.float8e4
I16 = mybir.dt.int16
AF = mybir.ActivationFunctionType
ALU = mybir.AluOpType

NP_BF16 = np.dtype(ml_dtypes.bfloat16)
NP_FP8 = np.dtype(ml_dtypes.float8_e4m3)

N_CORES = 8
NUM_NODES = 2500
T = 4
EMB = 256
U = 64
DIM_A = 32
EMBED_SIZE = 512
NEIGH = 10
B = NUM_NODES * T
NUM_SMS = 64
MAX_REGION = 32
NSEG = NUM_SMS * MAX_REGION  # 2048
SEG_PER_CORE = NSEG // N_CORES  # 256
E_TOTAL = 131072
P = 128
NCHUNK = 20          # ceil(2500 / 128) node chunks
NODES_PAD = NCHUNK * P  # 2560
NG = 2               # tiles per phase-1 matmul group
ROWS = None          # phase-2 lne row count = TT * P (runtime)


def _wrap16(flat):
    """dma_gather index layout: idx i -> partition i%16, column i//16,
    replicated across the 8 partition groups (one per Q7 core)."""
    n = flat.shape[0]
    assert n % 16 == 0
    a = flat.reshape(n // 16, 16).T.astype(np.int16)  # [16, n//16]
    return np.tile(a, (8, 1))  # [128, n//16]


# ---------------------------------------------------------------------------
# walrus post-pass: CoreV3 codegen rejects >1 sem wait on a TPB_CTRL (Drain);
# split the excess onto injected wait-only drains placed just before.
# ---------------------------------------------------------------------------
def _split_drain_waits(nc, max_waits=1):
    for bb in nc.main_func.blocks:
        out = []
        for ins in bb.instructions:
            si = ins.sync_info
            if (
                type(ins).__name__ == "InstDrain"
                and si is not None
                and si.on_wait is not None
                and len(si.on_wait) > max_waits
            ):
                waits = list(si.on_wait)
                extra, keep = waits[:-max_waits], waits[-max_waits:]
                for i in range(0, len(extra), max_waits):
                    d = mybir.InstDrain(
                        name=nc.get_next_instruction_name(),
                        ins=[],
                        outs=[],
                        bass_is_fusable=False,
                    )
                    d.engine = ins.engine
                    d.sync_info = mybir.SyncInfo(
                        on_wait=extra[i : i + max_waits], on_update=[]
                    )
                    out.append(d)
                ins.sync_info = mybir.SyncInfo(
                    on_wait=keep, on_update=list(si.on_update or [])
                )
            out.append(ins)
        bb.instructions = out


# ---------------------------------------------------------------------------
# Phase-1 program builder
# ---------------------------------------------------------------------------
def build_phase1(TPC, reps=1, debug=False, variant="full"):
    assert TPC % NG == 0
    G = TPC // NG
    AGRP = NCHUNK * T * NG * P  # free elems of one A group slice (fp8)
    nc = bacc.Bacc("TRN2", debug=False)
    ttab = nc.dram_tensor("ttab", [P, NCHUNK * T * U], BF16, kind="ExternalInput")
    aarr = nc.dram_tensor("aarr", [P, G * AGRP], FP8, kind="ExternalInput")
    ntab = nc.dram_tensor("ntab", [NUM_NODES, EMB], BF16, kind="ExternalInput")
    neidx = nc.dram_tensor("neidx", [P, TPC * 8], I16, kind="ExternalInput")
    s1w = nc.dram_tensor("s1w", [U, TPC * DIM_A], BF16, kind="ExternalInput")
    s2w = nc.dram_tensor("s2w", [DIM_A, TPC], BF16, kind="ExternalInput")
    ww = nc.dram_tensor("ww", [U, TPC * EMB], BF16, kind="ExternalInput")
    lne_out = nc.dram_tensor("lne", [P, TPC * EMB], BF16, kind="ExternalOutput")
    if debug:
        dbg_ntet = nc.dram_tensor("dbg_ntet", [U, T * TPC * P], BF16, kind="ExternalOutput")
        dbg_h = nc.dram_tensor("dbg_h", [DIM_A, TPC * T * P], BF16, kind="ExternalOutput")
        dbg_ex = nc.dram_tensor("dbg_ex", [P, TPC * T], F32, kind="ExternalOutput")
        dbg_agg = nc.dram_tensor("dbg_agg", [P, TPC * U], BF16, kind="ExternalOutput")
        dbg_nen = nc.dram_tensor("dbg_nen", [P, TPC * EMB], F32, kind="ExternalOutput")
        dbg_neg = nc.dram_tensor("dbg_neg", [P, TPC * EMB], BF16, kind="ExternalOutput")

    with tile.TileContext(nc) as tc:
        with (
            tc.tile_pool(name="const", bufs=1) as cpool,
            tc.tile_pool(name="astream", bufs=2) as apool,
            tc.tile_pool(name="work", bufs=2) as wpool,
            tc.tile_pool(name="ps_nt", bufs=1, space="PSUM") as ps_nt,
            tc.tile_pool(name="ps_h", bufs=2, space="PSUM") as ps_h,
            tc.tile_pool(name="ps_lg", bufs=2, space="PSUM") as ps_lg,
            tc.tile_pool(name="ps_tp", bufs=1, space="PSUM") as ps_tp,
        ):
            identb = cpool.tile([P, P], BF16)
            make_identity(nc, identb[:])
            ttab_sb = cpool.tile([P, NCHUNK, T, U], BF16)
            HC = NCHUNK // 2
            for hh in range(2):
                nc.sync.dma_start(
                    out=ttab_sb[:, hh * HC : (hh + 1) * HC, :, :],
                    in_=ttab[:, hh * HC * T * U : (hh + 1) * HC * T * U],
                )
            s1_sb = cpool.tile([U, TPC * DIM_A], BF16)
            nc.sync.dma_start(out=s1_sb[:], in_=s1w[:])
            s2_sb = cpool.tile([DIM_A, TPC], BF16)
            nc.sync.dma_start(out=s2_sb[:], in_=s2w[:])
            w_sb = cpool.tile([U, TPC * EMB], BF16)
            nc.sync.dma_start(out=w_sb[:], in_=ww[:])
            ne_idx_sb = cpool.tile([P, TPC * 8], I16)
            nc.sync.dma_start(out=ne_idx_sb[:], in_=neidx[:])
            nen_all = cpool.tile([P, TPC, EMB], F32)
            ss_all = cpool.tile([P, TPC], F32)
            lne_sb = cpool.tile([P, TPC, EMB], BF16)

            ntet_all = cpool.tile([U, T, TPC * P], BF16)
            h_all = cpool.tile([DIM_A, TPC, T, P], BF16)
            nte_all = cpool.tile([P, TPC, T, U], BF16)
            att_all = cpool.tile([P, TPC, T], F32)
            ex_all = cpool.tile([P, TPC, T], F32)
            tmp_all = cpool.tile([P, TPC, T, U], BF16)
            agg_all = cpool.tile([P, TPC, U], BF16)

            with tc.For_i(0, reps, 1) if reps > 1 else contextlib.nullcontext():
                # node_embeddings gather (the only dma_gather left: 128*TPC
                # descriptors of 512B)
                ne_g = cpool.tile([P, TPC, EMB], BF16, tag="ne_g")
                if variant == "nogather":
                    nc.vector.memset(ne_g[:], 0.01)
                else:
                    nc.gpsimd.dma_gather(
                        ne_g[:], ntab[:], ne_idx_sb[:], TPC * P, TPC * P, EMB,
                        single_packet=False,
                    )

                def stage_a(g):
                    a_hf = []
                    for hh in range(2):
                        ah = apool.tile(
                            [P, NCHUNK // 2, T, NG * P], FP8, tag=f"a{hh}"
                        )
                        nc.sync.dma_start(
                            out=ah[:],
                            in_=aarr[
                                :,
                                g * AGRP + hh * (AGRP // 2) : g * AGRP
                                + (hh + 1) * (AGRP // 2),
                            ],
                        )
                        a_hf.append(ah)
                    nt_ps = ps_nt.tile([U, T, NG * P], F32, tag="nt")
                    # t-outer: PSUM accumulation windows must be sequential
                    # per region — interleaving start/stop groups within one
                    # tile corrupts the accumulation
                    for t in range(T):
                        for c in range(NCHUNK):
                            nc.tensor.matmul(
                                out=nt_ps[:, t, :],
                                lhsT=ttab_sb[:, c, t, :],
                                rhs=a_hf[c // (NCHUNK // 2)][
                                    :, c % (NCHUNK // 2), t, :
                                ],
                                start=(c == 0),
                                stop=(c == NCHUNK - 1),
                                skip_group_check=True,
                            )
                    nc.scalar.activation(
                        ntet_all[:, :, g * NG * P : (g + 1) * NG * P],
                        nt_ps[:], AF.Copy,
                    )

                def pass1(g):
                    # h matmuls + tanh; nte transposes (inputs: ntet_all[g])
                    for j in range(g * NG, (g + 1) * NG):
                        sl = slice(j * P, (j + 1) * P)
                        h_ps = ps_h.tile([DIM_A, T, P], F32, tag="h_ps")
                        for t in range(T):
                            nc.tensor.matmul(
                                out=h_ps[:, t, :],
                                lhsT=s1_sb[:, j * DIM_A : (j + 1) * DIM_A],
                                rhs=ntet_all[:, t, sl],
                                start=True,
                                stop=True,
                                skip_group_check=True,
                            )
                        nc.scalar.activation(h_all[:, j], h_ps[:], AF.Tanh)
                        tp_ps = ps_tp.tile([P, T * U], BF16, tag="tp")
                        for t in range(T):
                            nc.tensor.transpose(
                                tp_ps[:, t * U : (t + 1) * U],
                                ntet_all[:, t, sl],
                                identb[:U, :U],
                            )
                        nc.vector.tensor_copy(out=nte_all[:, j], in_=tp_ps[:])

                def pass2(g):
                    # logits + batched softmax for the group's tiles. No max
                    # subtraction: |logits| <= 32 max|s2| stays far from f32
                    # exp range, and softmax(x) == softmax(x - max) exactly.
                    gs = slice(g * NG, (g + 1) * NG)
                    for j in range(g * NG, (g + 1) * NG):
                        lgd = ps_lg.tile([P, EMB], F32, tag="lgdl")
                        for t in range(T):
                            nc.tensor.matmul(
                                out=lgd[:, t : t + 1],
                                lhsT=h_all[:, j, t, :],
                                rhs=s2_sb[:, j : j + 1],
                                start=True,
                                stop=True,
                                skip_group_check=True,
                            )
                        nc.scalar.activation(ex_all[:, j], lgd[:, 0:T], AF.Exp)
                    ssum = wpool.tile([P, NG], F32, tag="ssum")
                    nc.vector.tensor_reduce(
                        out=ssum[:], in_=ex_all[:, gs],
                        axis=mybir.AxisListType.X, op=ALU.add,
                    )
                    rs = wpool.tile([P, NG], F32, tag="rs")
                    nc.vector.reciprocal(rs[:], ssum[:])
                    nc.vector.tensor_tensor(
                        out=att_all[:, gs],
                        in0=ex_all[:, gs],
                        in1=rs[:, :, None].to_broadcast([P, NG, T]),
                        op=ALU.mult,
                    )

                def pass3(g):
                    # batched weighted aggregation, then per-tile transpose +
                    # delta matmul + nen
                    gs = slice(g * NG, (g + 1) * NG)
                    with nc.allow_low_precision(
                        reason="bf16 staging of attention-weighted sums"
                    ):
                        nc.vector.tensor_tensor(
                            out=tmp_all[:, gs],
                            in0=nte_all[:, gs],
                            in1=att_all[:, gs][:, :, :, None].to_broadcast(
                                [P, NG, T, U]
                            ),
                            op=ALU.mult,
                        )
                        nc.vector.tensor_reduce(
                            out=agg_all[:, gs],
                            in_=tmp_all[:, gs].rearrange("p j t u -> p j u t"),
                            axis=mybir.AxisListType.X, op=ALU.add,
                        )
                    for j in range(g * NG, (g + 1) * NG):
                        at_ps = ps_tp.tile([P, T * U], BF16, tag="tp")
                        nc.tensor.transpose(
                            at_ps[:U, 0:P], agg_all[:, j], identb[:]
                        )
                        aggT = wpool.tile([U, P], BF16, tag="aggT")
                        nc.vector.tensor_copy(out=aggT[:], in_=at_ps[:U, 0:P])
                        dl_ps = ps_lg.tile([P, EMB], F32, tag="lgdl")
                        nc.tensor.matmul(
                            out=dl_ps[:],
                            lhsT=aggT[:],
                            rhs=w_sb[:, j * EMB : (j + 1) * EMB],
                            start=True,
                            stop=True,
                            skip_group_check=True,
                        )
                        nc.vector.tensor_add(
                            out=nen_all[:, j, :], in0=ne_g[:, j, :], in1=dl_ps[:]
                        )
                        sq = wpool.tile([P, EMB], F32, tag="sq")
                        nc.scalar.activation(
                            sq[:], nen_all[:, j, :], AF.Square,
                            accum_out=ss_all[:, j : j + 1],
                        )

                # software-pipelined schedule: stage A of group g overlaps
                # passes of earlier groups so PE rarely waits on ACT/DVE
                for g in range(G + 3):
                    if g < G:
                        stage_a(g)
                    if variant == "stageA":
                        continue
                    if 0 <= g - 1 < G:
                        pass1(g - 1)
                    if 0 <= g - 2 < G:
                        pass2(g - 2)
                    if 0 <= g - 3 < G:
                        pass3(g - 3)

                # ---- batched normalize: inv = ss^-0.5 on DVE keeps every
                # phase-1 ACT func inside one table set (no reloads)
                if variant == "stageA":
                    nc.vector.memset(lne_sb[:], 0.0)
                if variant != "stageA":
                    # inv = rsqrt(ss) via bit-trick seed + 2 Newton steps —
                    # all on DVE, so phase 1 never switches the ACT func table
                    # (tanh/exp/square/copy live in one set; sqrt does not).
                    ssc = wpool.tile([P, TPC], F32, tag="ssc")
                    nc.vector.tensor_scalar_max(ssc[:], ss_all[:], 1e-24)
                    ssh = wpool.tile([P, TPC], F32, tag="ssh")
                    nc.vector.tensor_scalar_mul(ssh[:], ssc[:], 0.5)
                    inv = wpool.tile([P, TPC], F32, tag="inv")
                    iv = inv[:].bitcast(mybir.dt.int32)
                    nc.vector.tensor_scalar(
                        out=iv, in0=ssc[:].bitcast(mybir.dt.int32),
                        scalar1=1, scalar2=None, op0=ALU.logical_shift_right,
                    )
                    nc.vector.tensor_scalar(
                        out=iv, in0=iv, scalar1=-1, scalar2=0x5f3759df,
                        op0=ALU.mult, op1=ALU.add,
                    )
                    tny = wpool.tile([P, TPC], F32, tag="tny")
                    for _ in range(2):
                        nc.vector.tensor_tensor(
                            out=tny[:], in0=inv[:], in1=inv[:], op=ALU.mult
                        )
                        nc.vector.tensor_tensor(
                            out=tny[:], in0=tny[:], in1=ssh[:], op=ALU.mult
                        )
                        nc.vector.tensor_scalar(
                            out=tny[:], in0=tny[:], scalar1=-1.0, scalar2=1.5,
                            op0=ALU.mult, op1=ALU.add,
                        )
                        nc.vector.tensor_tensor(
                            out=inv[:], in0=inv[:], in1=tny[:], op=ALU.mult
                        )
                    with nc.allow_low_precision(reason="bf16 output rows"):
                        nc.vector.tensor_tensor(
                            out=lne_sb[:],
                            in0=nen_all[:],
                            in1=inv[:, :, None].to_broadcast([P, TPC, EMB]),
                            op=ALU.mult,
                        )
                nc.sync.dma_start(out=lne_out[:], in_=lne_sb[:])
                if debug:
                    nc.sync.dma_start(out=dbg_ntet[:], in_=ntet_all[:])
                    nc.sync.dma_start(out=dbg_h[:], in_=h_all[:])
                    nc.sync.dma_start(out=dbg_ex[:], in_=ex_all[:])
                    nc.sync.dma_start(out=dbg_agg[:], in_=agg_all[:])
                    nc.sync.dma_start(out=dbg_nen[:], in_=nen_all[:])
                    nc.sync.dma_start(out=dbg_neg[:], in_=ne_g[:])

    nc.compile()
    _split_drain_waits(nc)
    return nc


# ---------------------------------------------------------------------------
# Phase-2 program builder
# ---------------------------------------------------------------------------
def build_phase2(n_rowchunk, reps=1, debug=False):
    """n_rowchunk: lne row chunks of 128 (TT*128 rows total).

    seg_acc[seg, e] = sum_row C[row, seg] * lne[row, e]; the segment MEAN's
    1/count scale cancels inside the subsequent L2 normalize (zero-count
    segments stay zero either way), so no recip pass is needed.
    """
    LPIECE = 8   # lne stream pieces
    CPIECE = 4   # C stream pieces
    assert LPIECE == 2 * CPIECE
    assert n_rowchunk % LPIECE == 0 and n_rowchunk % CPIECE == 0
    LC = n_rowchunk // LPIECE
    CC = n_rowchunk // CPIECE
    nc = bacc.Bacc("TRN2", debug=False)
    lne = nc.dram_tensor("lne", [P, n_rowchunk * EMB], BF16, kind="ExternalInput")
    carr = nc.dram_tensor("carr", [P, n_rowchunk * 2 * P], FP8, kind="ExternalInput")
    fcwt = nc.dram_tensor("fcwt", [EMB, EMBED_SIZE], BF16, kind="ExternalInput")
    fcb = nc.dram_tensor("fcb", [P, EMBED_SIZE], F32, kind="ExternalInput")
    out = nc.dram_tensor("out", [SEG_PER_CORE, EMBED_SIZE], F32, kind="ExternalOutput")
    if debug:
        dbg_acc = nc.dram_tensor("dbg_acc", [P, 2 * EMB], F32, kind="ExternalOutput")
        dbg_smn = nc.dram_tensor("dbg_smn", [P, 2 * EMB], F32, kind="ExternalOutput")
        dbg_xx = nc.dram_tensor("dbg_xx", [P, 2 * EMBED_SIZE], F32, kind="ExternalOutput")

    with tile.TileContext(nc) as tc:
        with (
            tc.tile_pool(name="const", bufs=1) as cpool,
            tc.tile_pool(name="stream", bufs=1) as spool,
            tc.tile_pool(name="work", bufs=2) as wpool,
            tc.tile_pool(name="ps_acc", bufs=1, space="PSUM") as ps_acc,
            tc.tile_pool(name="ps_tp", bufs=2, space="PSUM") as ps_tp,
            tc.tile_pool(name="ps_fc", bufs=2, space="PSUM") as ps_fc,
        ):
            identb = cpool.tile([P, P], BF16)
            make_identity(nc, identb[:])
            eps8 = cpool.tile([P, 1], F32)
            nc.vector.memset(eps8[:], 1e-8)

            with tc.For_i(0, reps, 1) if reps > 1 else contextlib.nullcontext():
                # interleave lne/C piece loads roughly in consumption order
                lne_sb = [None] * LPIECE
                c_sb = [None] * CPIECE
                for ci in range(CPIECE):
                    li = 2 * ci
                    tl = spool.tile([P, LC, EMB], BF16, tag=f"lne{li}")
                    nc.sync.dma_start(
                        out=tl[:],
                        in_=lne[:, li * LC * EMB : (li + 1) * LC * EMB],
                    )
                    lne_sb[li] = tl
                    tcl = spool.tile([P, CC, 2, P], FP8, tag=f"c{ci}")
                    nc.sync.dma_start(
                        out=tcl[:],
                        in_=carr[:, ci * CC * 2 * P : (ci + 1) * CC * 2 * P],
                    )
                    c_sb[ci] = tcl
                    li = 2 * ci + 1
                    tl = spool.tile([P, LC, EMB], BF16, tag=f"lne{li}")
                    nc.sync.dma_start(
                        out=tl[:],
                        in_=lne[:, li * LC * EMB : (li + 1) * LC * EMB],
                    )
                    lne_sb[li] = tl
                # const loads queue behind the stream (used only in the tail)
                fcwt0 = cpool.tile([P, EMBED_SIZE], BF16, tag="fcwt0")
                fcwt1 = cpool.tile([P, EMBED_SIZE], BF16, tag="fcwt1")
                fcwt_sb = [fcwt0, fcwt1]
                for i in range(2):
                    nc.sync.dma_start(
                        out=fcwt_sb[i][:], in_=fcwt[i * P : (i + 1) * P, :]
                    )
                fcb_sb = cpool.tile([P, EMBED_SIZE], F32, tag="fcb")
                nc.sync.dma_start(out=fcb_sb[:], in_=fcb[:])

                acc0 = ps_acc.tile([P, EMB], F32, tag="acc0")
                acc1 = ps_acc.tile([P, EMB], F32, tag="acc1")
                acc = [acc0, acc1]
                for c in range(n_rowchunk):
                    for half in range(2):
                        nc.tensor.matmul(
                            out=acc[half][:],
                            lhsT=c_sb[c // CC][:, c % CC, half, :],
                            rhs=lne_sb[c // LC][:, c % LC, :],
                            start=(c == 0),
                            stop=(c == n_rowchunk - 1),
                            skip_group_check=True,
                        )
                # ---- normalize (scale cancels; no mean division needed)
                smf = wpool.tile([P, 2, EMB], F32, tag="smf")
                for half in range(2):
                    nc.scalar.activation(smf[:, half, :], acc[half][:], AF.Copy)
                sq2 = wpool.tile([P, 2, EMB], F32, tag="sq2")
                nc.vector.tensor_tensor(
                    out=sq2[:], in0=smf[:], in1=smf[:], op=ALU.mult
                )
                ss2 = wpool.tile([P, 2], F32, tag="ss2")
                nc.vector.tensor_reduce(
                    out=ss2[:], in_=sq2[:], axis=mybir.AxisListType.X, op=ALU.add
                )
                nrm2 = wpool.tile([P, 2], F32, tag="nrm2")
                nc.scalar.activation(nrm2[:], ss2[:], AF.Sqrt)
                nc.vector.tensor_scalar_max(nrm2[:], nrm2[:], 1e-12)
                inv2 = wpool.tile([P, 2], F32, tag="inv2")
                nc.vector.reciprocal(inv2[:], nrm2[:])
                smn = wpool.tile([P, 2, EMB], BF16, tag="smn")
                with nc.allow_low_precision(reason="bf16 normalized rows"):
                    nc.vector.tensor_tensor(
                        out=smn[:], in0=smf[:],
                        in1=inv2[:, :, None].to_broadcast([P, 2, EMB]),
                        op=ALU.mult,
                    )
                # ---- transpose smn -> smnT [emb-half][128, (seg-half, s)]
                smnT = []
                for eh in range(2):
                    tp = ps_tp.tile([P, 2 * P], BF16, tag="tp")
                    for half in range(2):
                        nc.tensor.transpose(
                            tp[:, half * P : (half + 1) * P],
                            smn[:, half, eh * P : (eh + 1) * P],
                            identb[:],
                        )
                    st = wpool.tile([P, 2 * P], BF16, tag=f"smnT{eh}")
                    nc.vector.tensor_copy(out=st[:], in_=tp[:])
                    smnT.append(st)
                # ---- FC + bias + batched l2norm
                xx = wpool.tile([P, 2, EMBED_SIZE], F32, tag="xx")
                for m in range(2):
                    fc_ps = ps_fc.tile([P, EMBED_SIZE], F32, tag="fc_ps")
                    for kh in range(2):
                        nc.tensor.matmul(
                            out=fc_ps[:],
                            lhsT=smnT[kh][:, m * P : (m + 1) * P],
                            rhs=fcwt_sb[kh][:],
                            start=(kh == 0),
                            stop=(kh == 1),
                            skip_group_check=True,
                        )
                    nc.vector.tensor_add(
                        out=xx[:, m, :], in0=fc_ps[:], in1=fcb_sb[:]
                    )
                sq3 = wpool.tile([P, 2, EMBED_SIZE], F32, tag="sq3")
                nc.vector.tensor_tensor(
                    out=sq3[:], in0=xx[:], in1=xx[:], op=ALU.mult
                )
                ss3 = wpool.tile([P, 2], F32, tag="ss3")
                nc.vector.tensor_reduce(
                    out=ss3[:], in_=sq3[:], axis=mybir.AxisListType.X, op=ALU.add
                )
                nrm3 = wpool.tile([P, 2], F32, tag="nrm3")
                # n = sqrt(ss + 1e-8) + 1e-8
                nc.scalar.activation(nrm3[:], ss3[:], AF.Sqrt, bias=eps8[:, 0:1])
                nc.vector.tensor_scalar_add(nrm3[:], nrm3[:], 1e-8)
                inv3 = wpool.tile([P, 2], F32, tag="inv3")
                nc.vector.reciprocal(inv3[:], nrm3[:])
                res = wpool.tile([P, 2, EMBED_SIZE], F32, tag="res")
                nc.vector.tensor_tensor(
                    out=res[:], in0=xx[:],
                    in1=inv3[:, :, None].to_broadcast([P, 2, EMBED_SIZE]),
                    op=ALU.mult,
                )
                for m in range(2):
                    nc.sync.dma_start(
                        out=out[m * P : (m + 1) * P, :], in_=res[:, m, :]
                    )
                if debug:
                    nc.sync.dma_start(out=dbg_acc[:], in_=smf[:])
                    dbg_smn_sb = wpool.tile([P, 2, EMB], F32, tag="dbgsmn")
                    nc.vector.tensor_copy(out=dbg_smn_sb[:], in_=smn[:])
                    nc.sync.dma_start(out=dbg_smn[:], in_=dbg_smn_sb[:])
                    nc.sync.dma_start(out=dbg_xx[:], in_=xx[:])

    nc.compile()
    _split_drain_waits(nc)
    return nc


# ---------------------------------------------------------------------------
# Host-side orchestration
# ---------------------------------------------------------------------------
def _phase1_prep(train_inputs, train_types, node_neigh):
    order = np.argsort(train_types, kind="stable")
    ts = train_types[order]
    tiles_s, tiles_t = [], []
    for t in range(T):
        idx_t = order[ts == t]
        if len(idx_t) == 0:
            continue
        n_tiles = -(-len(idx_t) // P)
        padded = np.concatenate(
            [idx_t, np.repeat(idx_t[-1:], n_tiles * P - len(idx_t))]
        )
        for jj in range(n_tiles):
            tiles_s.append(padded[jj * P : (jj + 1) * P])
            tiles_t.append(t)
    while len(tiles_s) % (N_CORES * NG):
        tiles_s.append(tiles_s[-1])
        tiles_t.append(tiles_t[-1])
    sample_mat = np.stack(tiles_s)  # [TT, 128]
    tile_type = np.asarray(tiles_t)
    TT = sample_mat.shape[0]
    TPC = TT // N_CORES

    flat = sample_mat.reshape(-1)
    slot_of_sample = np.zeros(B, np.int64)
    slot_of_sample[flat[::-1]] = np.arange(TT * P)[::-1]
    return sample_mat, tile_type, TPC, slot_of_sample


def _phase1_inmaps(inputs, sample_mat, tile_type, TPC):
    node_embeddings = np.asarray(inputs["node_embeddings"], np.float32)
    node_type_embeddings = np.asarray(inputs["node_type_embeddings"], np.float32)
    trans_weights = np.asarray(inputs["trans_weights"], np.float32)
    trans_weights_s1 = np.asarray(inputs["trans_weights_s1"], np.float32)
    trans_weights_s2 = np.asarray(inputs["trans_weights_s2"], np.float32)
    train_inputs = np.asarray(inputs["train_inputs"])
    node_neigh = np.asarray(inputs["node_neigh"])
    G = TPC // NG

    # ttab_r[p, c, t, u] = NTE[c*128+p, t, u]  (node-chunked, zero padded)
    nte_pad = np.zeros((NODES_PAD, T, U), np.float32)
    nte_pad[:NUM_NODES] = node_type_embeddings
    ttab_r = np.ascontiguousarray(
        nte_pad.reshape(NCHUNK, P, T, U).transpose(1, 0, 2, 3)
    ).reshape(P, NCHUNK * T * U).astype(NP_BF16)

    ntab = node_embeddings.astype(NP_BF16)

    in_maps = []
    for k in range(N_CORES):
        smp = sample_mat[k * TPC : (k + 1) * TPC]  # [TPC, 128]
        ct = tile_type[k * TPC : (k + 1) * TPC]  # [TPC]
        ne_idx = _wrap16(train_inputs[smp].reshape(-1))
        # A[p, g, c, t, s_local] neighbor count matrix
        refs = node_neigh[smp]  # [TPC, 128, 4, 10] node ids
        a_f = np.zeros((P, G, NCHUNK, T, NG * P), np.float32)
        jj = np.arange(TPC)[:, None, None, None]
        ss = np.arange(P)[None, :, None, None]
        tt = np.arange(T)[None, None, :, None]
        p_idx = (refs % P).astype(np.int64)
        c_idx = (refs // P).astype(np.int64)
        g_idx = np.broadcast_to(jj // NG, refs.shape)
        s_idx = np.broadcast_to((jj % NG) * P + ss, refs.shape)
        t_idx = np.broadcast_to(tt, refs.shape)
        np.add.at(a_f, (p_idx, g_idx, c_idx, t_idx, s_idx), 1.0)
        aarr = a_f.reshape(P, -1).astype(NP_FP8)

        s1_all = np.ascontiguousarray(
            trans_weights_s1[ct].transpose(1, 0, 2).reshape(U, TPC * DIM_A)
        ).astype(NP_BF16)
        w_all = np.ascontiguousarray(
            trans_weights[ct].transpose(1, 0, 2).reshape(U, TPC * EMB)
        ).astype(NP_BF16)
        s2_blk = np.ascontiguousarray(
            trans_weights_s2[ct][:, :, 0].T
        ).astype(NP_BF16)  # [32, TPC]
        in_maps.append(
            {
                "ttab": ttab_r,
                "aarr": aarr,
                "ntab": ntab,
                "neidx": ne_idx,
                "s1w": s1_all,
                "s2w": s2_blk,
                "ww": w_all,
            }
        )
    return in_maps


def _phase2_prep(region_index, region_segment_ids, slot_of_sample, n_rows):
    """Per-core fp8 count matrix C[p, chunk, half, seg_local]."""
    seg_ids = np.asarray(region_segment_ids).astype(np.int64)
    rows = slot_of_sample[np.asarray(region_index).astype(np.int64)]
    n_rowchunk = n_rows // P

    # dense counts [n_rows, NSEG] via bincount on linear index
    lin = rows * NSEG + seg_ids
    dense = np.bincount(lin, minlength=n_rows * NSEG).astype(np.float32)
    dense = dense.reshape(n_rowchunk, P, NSEG)  # [chunk, p, seg]

    carr_l = []
    for k in range(N_CORES):
        sl = dense[:, :, k * SEG_PER_CORE : (k + 1) * SEG_PER_CORE]
        # [p, chunk, half, seg_local]
        ck = np.ascontiguousarray(
            sl.reshape(n_rowchunk, P, 2, P).transpose(1, 0, 2, 3)
        ).reshape(P, -1).astype(NP_FP8)
        carr_l.append(ck)
    return carr_l


def _phase2_inmaps(inputs, lne_bf, carr_l):
    fc_w = np.asarray(inputs["fc_w"], np.float32)
    fc_b = np.asarray(inputs["fc_b"], np.float32)
    fcwt = np.ascontiguousarray(fc_w.T).astype(NP_BF16)  # [256, 512]
    fcb = np.broadcast_to(fc_b[None, :], (P, EMBED_SIZE)).astype(np.float32).copy()
    in_maps = []
    for k in range(N_CORES):
        in_maps.append(
            {
                "lne": lne_bf,
                "carr": carr_l[k],
                "fcwt": fcwt,
                "fcb": fcb,
            }
        )
    return in_maps


def _run_spmd_retry(nc, in_maps, retries=3, delay=45.0):
    """The axon-tunneled device occasionally reports a transient
    UNAVAILABLE/unrecoverable state; back off and retry."""
    import time as _time

    last = None
    for attempt in range(retries):
        try:
            return run_bass_kernel_spmd(nc, in_maps, list(range(N_CORES)))
        except Exception as e:  # jax.errors.JaxRuntimeError and friends
            last = e
            if attempt + 1 < retries:
                _time.sleep(delay)
    raise last


_P1_CACHE = {}
_P2_CACHE = {}


def kernel(**inputs) -> np.ndarray:
    train_inputs = np.asarray(inputs["train_inputs"])
    train_types = np.asarray(inputs["train_types"])
    node_neigh = np.asarray(inputs["node_neigh"])
    num_sms = int(inputs["num_sms"])
    max_region = int(inputs["max_region"])
    assert num_sms * max_region == NSEG

    sample_mat, tile_type, TPC, slot_of_sample = _phase1_prep(
        train_inputs, train_types, node_neigh
    )
    TT = sample_mat.shape[0]

    if TPC not in _P1_CACHE:
        _P1_CACHE[TPC] = build_phase1(TPC)
    nc1 = _P1_CACHE[TPC]
    in_maps1 = _phase1_inmaps(inputs, sample_mat, tile_type, TPC)
    res1 = _run_spmd_retry(nc1, in_maps1).results

    # relay: lne row at global_row = tile*128 + p, laid out [p, chunk, emb]
    n_rows = TT * P
    lne_bf = np.empty((P, TT, EMB), NP_BF16)
    for k in range(N_CORES):
        lne_bf[:, k * TPC : (k + 1) * TPC, :] = (
            np.asarray(res1[k]["lne"]).reshape(P, TPC, EMB)
        )
    lne_bf = lne_bf.reshape(P, TT * EMB)

    carr_l = _phase2_prep(
        inputs["region_index"], inputs["region_segment_ids"], slot_of_sample,
        n_rows,
    )
    n_rowchunk = n_rows // P
    if n_rowchunk not in _P2_CACHE:
        _P2_CACHE[n_rowchunk] = build_phase2(n_rowchunk)
    nc2 = _P2_CACHE[n_rowchunk]
    in_maps2 = _phase2_inmaps(inputs, lne_bf, carr_l)
    res2 = _run_spmd_retry(nc2, in_maps2).results

    out = np.concatenate(
        [np.asarray(res2[k]["out"], np.float32) for k in range(N_CORES)], axis=0
    )
    return out.reshape(num_sms, max_region, EMBED_SIZE)


# revision 26
# speedup vs baseline: 12.5107x; 3.3040x over previous
"""GATNE model (attention over edge types + ragged segment-mean + FC) on 8
Trainium2 NeuronCores via Bass/Tile.

v2 strategy (self-contained; hardcoded for the spec shapes). The v1 kernel
was descriptor-bound: 52k dma_gather descriptors in phase 1 and 18k in
phase 2 dominate on HW (SWDGE desc-gen on the Q7). v2 eliminates nearly all
gather descriptors by turning both irregular reductions into dense matmuls
against host-built fp8 count matrices (integer index preprocessing only —
all float math stays on device):

  Phase 1 (SPMD, data-parallel over B): samples sorted by train_type into
  128-sample type-homogeneous tiles. The neighbor gather-sum becomes
  ntet[u, (t,s)] = sum_chunks ttab_chunk[node,u]^T @ A_chunk[node, (t,s)]
  with ttab (bf16) SBUF-resident and A (fp8 counts, 0/1/2...) streamed.
  Attention (PE matmuls + ACT tanh/exp), per-sample trans_weights matmul,
  and a batched row-L2-normalize (single Sqrt -> one act-table switch per
  iteration instead of 18). Output: bf16 embedding table shard per core.
  Host relays the 8 shards into one table (free between phases).

  Phase 2 (SPMD, sharded by segment blocks: 256 segments per core): the
  ragged segment-sum becomes seg_acc[seg, e] = C^T[row, seg]^T @ lne[row, e]
  streaming both the bf16 lne table (5.2 MB) and the fp8 count matrix C
  (2.6 MB) — zero gather descriptors. Then mean + normalize + FC + l2norm.
"""
import contextlib
import sys

sys.path.insert(0, "/opt/trn_rl_repo")

import numpy as np
import ml_dtypes

import concourse.bacc as bacc
import concourse.bass as bass
import concourse.mybir as mybir
import concourse.tile as tile
from concourse.bass_utils import run_bass_kernel_spmd
from concourse.masks import make_identity

F32 = mybir.dt.float32
BF16 = mybir.dt.bfloat16
FP8 = mybir.dt.float8e4
I16 = mybir.dt.int16
AF = mybir.ActivationFunctionType
ALU = mybir.AluOpType

NP_BF16 = np.dtype(ml_dtypes.bfloat16)
NP_FP8 = np.dtype(ml_dtypes.float8_e4m3)

N_CORES = 8
NUM_NODES = 2500
T = 4
EMB = 256
U = 64
DIM_A = 32
EMBED_SIZE = 512
NEIGH = 10
B = NUM_NODES * T
NUM_SMS = 64
MAX_REGION = 32
NSEG = NUM_SMS * MAX_REGION  # 2048
SEG_PER_CORE = NSEG // N_CORES  # 256
E_TOTAL = 131072
P = 128
NCHUNK = 20          # ceil(2500 / 128) node chunks
NODES_PAD = NCHUNK * P  # 2560
NG = 2               # tiles per phase-1 matmul group
ROWS = None          # phase-2 lne row count = TT * P (runtime)


def _wrap16(flat):
    """dma_gather index layout: idx i -> partition i%16, column i//16,
    replicated across the 8 partition groups (one per Q7 core)."""
    n = flat.shape[0]
    assert n % 16 == 0
    a = flat.reshape(n // 16, 16).T.astype(np.int16)  # [16, n//16]
    return np.tile(a, (8, 1))  # [128, n//16]


# ---------------------------------------------------------------------------
# walrus post-pass: CoreV3 codegen rejects >1 sem wait on a TPB_CTRL (Drain);
# split the excess onto injected wait-only drains placed just before.
# ---------------------------------------------------------------------------
def _split_drain_waits(nc, max_waits=1):
    for bb in nc.main_func.blocks:
        out = []
        for ins in bb.instructions:
            si = ins.sync_info
            if (
                type(ins).__name__ == "InstDrain"
                and si is not None
                and si.on_wait is not None
                and len(si.on_wait) > max_waits
            ):
                waits = list(si.on_wait)
                extra, keep = waits[:-max_waits], waits[-max_waits:]
                for i in range(0, len(extra), max_waits):
                    d = mybir.InstDrain(
                        name=nc.get_next_instruction_name(),
                        ins=[],
                        outs=[],
                        bass_is_fusable=False,
                    )
                    d.engine = ins.engine
                    d.sync_info = mybir.SyncInfo(
                        on_wait=extra[i : i + max_waits], on_update=[]
                    )
                    out.append(d)
                ins.sync_info = mybir.SyncInfo(
                    on_wait=keep, on_update=list(si.on_update or [])
                )
            out.append(ins)
        bb.instructions = out


# ---------------------------------------------------------------------------
# Phase-1 program builder
# ---------------------------------------------------------------------------
def build_phase1(TPC, reps=1, debug=False, variant="full"):
    assert TPC % NG == 0
    G = TPC // NG
    AGRP = NCHUNK * T * NG * P  # free elems of one A group slice (fp8)
    nc = bacc.Bacc("TRN2", debug=False)
    ttab = nc.dram_tensor("ttab", [P, NCHUNK * T * U], BF16, kind="ExternalInput")
    aarr = nc.dram_tensor("aarr", [P, G * AGRP], FP8, kind="ExternalInput")
    ntab = nc.dram_tensor("ntab", [NUM_NODES, EMB], BF16, kind="ExternalInput")
    neidx = nc.dram_tensor("neidx", [P, TPC * 8], I16, kind="ExternalInput")
    s1w = nc.dram_tensor("s1w", [U, TPC * DIM_A], BF16, kind="ExternalInput")
    s2w = nc.dram_tensor("s2w", [DIM_A, TPC], BF16, kind="ExternalInput")
    ww = nc.dram_tensor("ww", [U, TPC * EMB], BF16, kind="ExternalInput")
    lne_out = nc.dram_tensor("lne", [P, TPC * EMB], BF16, kind="ExternalOutput")
    if debug:
        dbg_ntet = nc.dram_tensor("dbg_ntet", [U, T * TPC * P], BF16, kind="ExternalOutput")
        dbg_h = nc.dram_tensor("dbg_h", [DIM_A, TPC * T * P], BF16, kind="ExternalOutput")
        dbg_ex = nc.dram_tensor("dbg_ex", [P, TPC * T], F32, kind="ExternalOutput")
        dbg_agg = nc.dram_tensor("dbg_agg", [P, TPC * U], BF16, kind="ExternalOutput")
        dbg_nen = nc.dram_tensor("dbg_nen", [P, TPC * EMB], F32, kind="ExternalOutput")
        dbg_neg = nc.dram_tensor("dbg_neg", [P, TPC * EMB], BF16, kind="ExternalOutput")

    with tile.TileContext(nc) as tc:
        with (
            tc.tile_pool(name="const", bufs=1) as cpool,
            tc.tile_pool(name="astream", bufs=2) as apool,
            tc.tile_pool(name="work", bufs=2) as wpool,
            tc.tile_pool(name="ps_nt", bufs=1, space="PSUM") as ps_nt,
            tc.tile_pool(name="ps_h", bufs=2, space="PSUM") as ps_h,
            tc.tile_pool(name="ps_lg", bufs=2, space="PSUM") as ps_lg,
            tc.tile_pool(name="ps_tp", bufs=1, space="PSUM") as ps_tp,
        ):
            identb = cpool.tile([P, P], BF16)
            make_identity(nc, identb[:])
            ttab_sb = cpool.tile([P, NCHUNK, T, U], BF16)
            HC = NCHUNK // 2
            for hh in range(2):
                nc.sync.dma_start(
                    out=ttab_sb[:, hh * HC : (hh + 1) * HC, :, :],
                    in_=ttab[:, hh * HC * T * U : (hh + 1) * HC * T * U],
                )
            s1_sb = cpool.tile([U, TPC * DIM_A], BF16)
            nc.sync.dma_start(out=s1_sb[:], in_=s1w[:])
            s2_sb = cpool.tile([DIM_A, TPC], BF16)
            nc.sync.dma_start(out=s2_sb[:], in_=s2w[:])
            w_sb = cpool.tile([U, TPC * EMB], BF16)
            nc.sync.dma_start(out=w_sb[:], in_=ww[:])
            ne_idx_sb = cpool.tile([P, TPC * 8], I16)
            nc.sync.dma_start(out=ne_idx_sb[:], in_=neidx[:])
            nen_all = cpool.tile([P, TPC, EMB], F32)
            ss_all = cpool.tile([P, TPC], F32)
            lne_sb = cpool.tile([P, TPC, EMB], BF16)

            ntet_all = cpool.tile([U, T, TPC * P], BF16)
            h_all = cpool.tile([DIM_A, TPC, T, P], BF16)
            nte_all = cpool.tile([P, TPC, T, U], BF16)
            att_all = cpool.tile([P, TPC, T], F32)
            ex_all = cpool.tile([P, TPC, T], F32)
            tmp_all = cpool.tile([P, TPC, T, U], BF16)
            agg_all = cpool.tile([P, TPC, U], BF16)

            with tc.For_i(0, reps, 1) if reps > 1 else contextlib.nullcontext():
                # node_embeddings gather (the only dma_gather left: 128*TPC
                # descriptors of 512B)
                ne_g = cpool.tile([P, TPC, EMB], BF16, tag="ne_g")
                if variant == "nogather":
                    nc.vector.memset(ne_g[:], 0.01)
                else:
                    nc.gpsimd.dma_gather(
                        ne_g[:], ntab[:], ne_idx_sb[:], TPC * P, TPC * P, EMB,
                        single_packet=False,
                    )

                def stage_a(g):
                    a_hf = []
                    for hh in range(2):
                        ah = apool.tile(
                            [P, NCHUNK // 2, T, NG * P], FP8, tag=f"a{hh}"
                        )
                        nc.sync.dma_start(
                            out=ah[:],
                            in_=aarr[
                                :,
                                g * AGRP + hh * (AGRP // 2) : g * AGRP
                                + (hh + 1) * (AGRP // 2),
                            ],
                        )
                        a_hf.append(ah)
                    nt_ps = ps_nt.tile([U, T, NG * P], F32, tag="nt")
                    # t-outer: PSUM accumulation windows must be sequential
                    # per region — interleaving start/stop groups within one
                    # tile corrupts the accumulation
                    for t in range(T):
                        for c in range(NCHUNK):
                            nc.tensor.matmul(
                                out=nt_ps[:, t, :],
                                lhsT=ttab_sb[:, c, t, :],
                                rhs=a_hf[c // (NCHUNK // 2)][
                                    :, c % (NCHUNK // 2), t, :
                                ],
                                start=(c == 0),
                                stop=(c == NCHUNK - 1),
                                skip_group_check=True,
                            )
                    nc.scalar.activation(
                        ntet_all[:, :, g * NG * P : (g + 1) * NG * P],
                        nt_ps[:], AF.Copy,
                    )

                def pass1(g):
                    # h matmuls + tanh; nte transposes (inputs: ntet_all[g])
                    for j in range(g * NG, (g + 1) * NG):
                        sl = slice(j * P, (j + 1) * P)
                        h_ps = ps_h.tile([DIM_A, T, P], F32, tag="h_ps")
                        for t in range(T):
                            nc.tensor.matmul(
                                out=h_ps[:, t, :],
                                lhsT=s1_sb[:, j * DIM_A : (j + 1) * DIM_A],
                                rhs=ntet_all[:, t, sl],
                                start=True,
                                stop=True,
                                skip_group_check=True,
                            )
                        nc.scalar.activation(h_all[:, j], h_ps[:], AF.Tanh)
                        tp_ps = ps_tp.tile([P, T * U], BF16, tag="tp")
                        for t in range(T):
                            nc.tensor.transpose(
                                tp_ps[:, t * U : (t + 1) * U],
                                ntet_all[:, t, sl],
                                identb[:U, :U],
                            )
                        nc.vector.tensor_copy(out=nte_all[:, j], in_=tp_ps[:])

                def pass2(g):
                    # logits + batched softmax for the group's tiles. No max
                    # subtraction: |logits| <= 32 max|s2| stays far from f32
                    # exp range, and softmax(x) == softmax(x - max) exactly.
                    gs = slice(g * NG, (g + 1) * NG)
                    for j in range(g * NG, (g + 1) * NG):
                        lgd = ps_lg.tile([P, EMB], F32, tag="lgdl")
                        for t in range(T):
                            nc.tensor.matmul(
                                out=lgd[:, t : t + 1],
                                lhsT=h_all[:, j, t, :],
                                rhs=s2_sb[:, j : j + 1],
                                start=True,
                                stop=True,
                                skip_group_check=True,
                            )
                        nc.scalar.activation(ex_all[:, j], lgd[:, 0:T], AF.Exp)
                    ssum = wpool.tile([P, NG], F32, tag="ssum")
                    nc.vector.tensor_reduce(
                        out=ssum[:], in_=ex_all[:, gs],
                        axis=mybir.AxisListType.X, op=ALU.add,
                    )
                    rs = wpool.tile([P, NG], F32, tag="rs")
                    nc.vector.reciprocal(rs[:], ssum[:])
                    nc.vector.tensor_tensor(
                        out=att_all[:, gs],
                        in0=ex_all[:, gs],
                        in1=rs[:, :, None].to_broadcast([P, NG, T]),
                        op=ALU.mult,
                    )

                def pass3(g):
                    # batched weighted aggregation, then per-tile transpose +
                    # delta matmul + nen
                    gs = slice(g * NG, (g + 1) * NG)
                    with nc.allow_low_precision(
                        reason="bf16 staging of attention-weighted sums"
                    ):
                        nc.vector.tensor_tensor(
                            out=tmp_all[:, gs],
                            in0=nte_all[:, gs],
                            in1=att_all[:, gs][:, :, :, None].to_broadcast(
                                [P, NG, T, U]
                            ),
                            op=ALU.mult,
                        )
                        nc.vector.tensor_reduce(
                            out=agg_all[:, gs],
                            in_=tmp_all[:, gs].rearrange("p j t u -> p j u t"),
                            axis=mybir.AxisListType.X, op=ALU.add,
                        )
                    for j in range(g * NG, (g + 1) * NG):
                        at_ps = ps_tp.tile([P, T * U], BF16, tag="tp")
                        nc.tensor.transpose(
                            at_ps[:U, 0:P], agg_all[:, j], identb[:]
                        )
                        aggT = wpool.tile([U, P], BF16, tag="aggT")
                        nc.vector.tensor_copy(out=aggT[:], in_=at_ps[:U, 0:P])
                        dl_ps = ps_lg.tile([P, EMB], F32, tag="lgdl")
                        nc.tensor.matmul(
                            out=dl_ps[:],
                            lhsT=aggT[:],
                            rhs=w_sb[:, j * EMB : (j + 1) * EMB],
                            start=True,
                            stop=True,
                            skip_group_check=True,
                        )
                        nc.vector.tensor_add(
                            out=nen_all[:, j, :], in0=ne_g[:, j, :], in1=dl_ps[:]
                        )
                        sq = wpool.tile([P, EMB], F32, tag="sq")
                        nc.scalar.activation(
                            sq[:], nen_all[:, j, :], AF.Square,
                            accum_out=ss_all[:, j : j + 1],
                        )

                # software-pipelined schedule: stage A of group g overlaps
                # passes of earlier groups so PE rarely waits on ACT/DVE
                for g in range(G + 3):
                    if g < G:
                        stage_a(g)
                    if variant == "stageA":
                        continue
                    if 0 <= g - 1 < G:
                        pass1(g - 1)
                    if 0 <= g - 2 < G:
                        pass2(g - 2)
                    if 0 <= g - 3 < G:
                        pass3(g - 3)

                # ---- batched normalize: inv = ss^-0.5 on DVE keeps every
                # phase-1 ACT func inside one table set (no reloads)
                if variant == "stageA":
                    nc.vector.memset(lne_sb[:], 0.0)
                if variant != "stageA":
                    # inv = rsqrt(ss) via bit-trick seed + 2 Newton steps —
                    # all on DVE, so phase 1 never switches the ACT func table
                    # (tanh/exp/square/copy live in one set; sqrt does not).
                    ssc = wpool.tile([P, TPC], F32, tag="ssc")
                    nc.vector.tensor_scalar_max(ssc[:], ss_all[:], 1e-24)
                    ssh = wpool.tile([P, TPC], F32, tag="ssh")
                    nc.vector.tensor_scalar_mul(ssh[:], ssc[:], 0.5)
                    inv = wpool.tile([P, TPC], F32, tag="inv")
                    iv = inv[:].bitcast(mybir.dt.int32)
                    nc.vector.tensor_scalar(
                        out=iv, in0=ssc[:].bitcast(mybir.dt.int32),
                        scalar1=1, scalar2=None, op0=ALU.logical_shift_right,
                    )
                    nc.vector.tensor_scalar(
                        out=iv, in0=iv, scalar1=-1, scalar2=0x5f3759df,
                        op0=ALU.mult, op1=ALU.add,
                    )
                    tny = wpool.tile([P, TPC], F32, tag="tny")
                    for _ in range(2):
                        nc.vector.tensor_tensor(
                            out=tny[:], in0=inv[:], in1=inv[:], op=ALU.mult
                        )
                        nc.vector.tensor_tensor(
                            out=tny[:], in0=tny[:], in1=ssh[:], op=ALU.mult
                        )
                        nc.vector.tensor_scalar(
                            out=tny[:], in0=tny[:], scalar1=-1.0, scalar2=1.5,
                            op0=ALU.mult, op1=ALU.add,
                        )
                        nc.vector.tensor_tensor(
                            out=inv[:], in0=inv[:], in1=tny[:], op=ALU.mult
                        )
                    with nc.allow_low_precision(reason="bf16 output rows"):
                        nc.vector.tensor_tensor(
                            out=lne_sb[:],
                            in0=nen_all[:],
                            in1=inv[:, :, None].to_broadcast([P, TPC, EMB]),
                            op=ALU.mult,
                        )
                nc.sync.dma_start(out=lne_out[:], in_=lne_sb[:])
                if debug:
                    nc.sync.dma_start(out=dbg_ntet[:], in_=ntet_all[:])
                    nc.sync.dma_start(out=dbg_h[:], in_=h_all[:])
                    nc.sync.dma_start(out=dbg_ex[:], in_=ex_all[:])
                    nc.sync.dma_start(out=dbg_agg[:], in_=agg_all[:])
                    nc.sync.dma_start(out=dbg_nen[:], in_=nen_all[:])
                    nc.sync.dma_start(out=dbg_neg[:], in_=ne_g[:])

    nc.compile()
    _split_drain_waits(nc)
    return nc


# ---------------------------------------------------------------------------
# Phase-2 program builder
# ---------------------------------------------------------------------------
def build_phase2(n_rowchunk, reps=1, debug=False):
    """n_rowchunk: lne row chunks of 128 (TT*128 rows total).

    seg_acc[seg, e] = sum_row C[row, seg] * lne[row, e]; the segment MEAN's
    1/count scale cancels inside the subsequent L2 normalize (zero-count
    segments stay zero either way), so no recip pass is needed.
    """
    LPIECE = 8   # lne stream pieces
    CPIECE = 4   # C stream pieces
    assert LPIECE == 2 * CPIECE
    assert n_rowchunk % LPIECE == 0 and n_rowchunk % CPIECE == 0
    LC = n_rowchunk // LPIECE
    CC = n_rowchunk // CPIECE
    nc = bacc.Bacc("TRN2", debug=False)
    lne = nc.dram_tensor("lne", [P, n_rowchunk * EMB], BF16, kind="ExternalInput")
    carr = nc.dram_tensor("carr", [P, n_rowchunk * 2 * P], FP8, kind="ExternalInput")
    fcwt = nc.dram_tensor("fcwt", [EMB, EMBED_SIZE], BF16, kind="ExternalInput")
    fcb = nc.dram_tensor("fcb", [P, EMBED_SIZE], F32, kind="ExternalInput")
    out = nc.dram_tensor("out", [SEG_PER_CORE, EMBED_SIZE], F32, kind="ExternalOutput")
    if debug:
        dbg_acc = nc.dram_tensor("dbg_acc", [P, 2 * EMB], F32, kind="ExternalOutput")
        dbg_smn = nc.dram_tensor("dbg_smn", [P, 2 * EMB], F32, kind="ExternalOutput")
        dbg_xx = nc.dram_tensor("dbg_xx", [P, 2 * EMBED_SIZE], F32, kind="ExternalOutput")

    with tile.TileContext(nc) as tc:
        with (
            tc.tile_pool(name="const", bufs=1) as cpool,
            tc.tile_pool(name="stream", bufs=1) as spool,
            tc.tile_pool(name="work", bufs=2) as wpool,
            tc.tile_pool(name="ps_acc", bufs=1, space="PSUM") as ps_acc,
            tc.tile_pool(name="ps_tp", bufs=2, space="PSUM") as ps_tp,
            tc.tile_pool(name="ps_fc", bufs=2, space="PSUM") as ps_fc,
        ):
            identb = cpool.tile([P, P], BF16)
            make_identity(nc, identb[:])
            eps8 = cpool.tile([P, 1], F32)
            nc.vector.memset(eps8[:], 1e-8)

            with tc.For_i(0, reps, 1) if reps > 1 else contextlib.nullcontext():
                # interleave lne/C piece loads roughly in consumption order
                lne_sb = [None] * LPIECE
                c_sb = [None] * CPIECE
                for ci in range(CPIECE):
                    li = 2 * ci
                    tl = spool.tile([P, LC, EMB], BF16, tag=f"lne{li}")
                    nc.sync.dma_start(
                        out=tl[:],
                        in_=lne[:, li * LC * EMB : (li + 1) * LC * EMB],
                    )
                    lne_sb[li] = tl
                    tcl = spool.tile([P, CC, 2, P], FP8, tag=f"c{ci}")
                    nc.sync.dma_start(
                        out=tcl[:],
                        in_=carr[:, ci * CC * 2 * P : (ci + 1) * CC * 2 * P],
                    )
                    c_sb[ci] = tcl
                    li = 2 * ci + 1
                    tl = spool.tile([P, LC, EMB], BF16, tag=f"lne{li}")
                    nc.sync.dma_start(
                        out=tl[:],
                        in_=lne[:, li * LC * EMB : (li + 1) * LC * EMB],
                    )
                    lne_sb[li] = tl
                # const loads queue behind the stream (used only in the tail)
                fcwt0 = cpool.tile([P, EMBED_SIZE], BF16, tag="fcwt0")
                fcwt1 = cpool.tile([P, EMBED_SIZE], BF16, tag="fcwt1")
                fcwt_sb = [fcwt0, fcwt1]
                for i in range(2):
                    nc.sync.dma_start(
                        out=fcwt_sb[i][:], in_=fcwt[i * P : (i + 1) * P, :]
                    )
                fcb_sb = cpool.tile([P, EMBED_SIZE], F32, tag="fcb")
                nc.sync.dma_start(out=fcb_sb[:], in_=fcb[:])

                acc0 = ps_acc.tile([P, EMB], F32, tag="acc0")
                acc1 = ps_acc.tile([P, EMB], F32, tag="acc1")
                acc = [acc0, acc1]
                for c in range(n_rowchunk):
                    for half in range(2):
                        nc.tensor.matmul(
                            out=acc[half][:],
                            lhsT=c_sb[c // CC][:, c % CC, half, :],
                            rhs=lne_sb[c // LC][:, c % LC, :],
                            start=(c == 0),
                            stop=(c == n_rowchunk - 1),
                            skip_group_check=True,
                        )
                # ---- normalize (scale cancels; no mean division needed)
                smf = wpool.tile([P, 2, EMB], F32, tag="smf")
                for half in range(2):
                    nc.scalar.activation(smf[:, half, :], acc[half][:], AF.Copy)
                sq2 = wpool.tile([P, 2, EMB], F32, tag="sq2")
                nc.vector.tensor_tensor(
                    out=sq2[:], in0=smf[:], in1=smf[:], op=ALU.mult
                )
                ss2 = wpool.tile([P, 2], F32, tag="ss2")
                nc.vector.tensor_reduce(
                    out=ss2[:], in_=sq2[:], axis=mybir.AxisListType.X, op=ALU.add
                )
                nrm2 = wpool.tile([P, 2], F32, tag="nrm2")
                nc.scalar.activation(nrm2[:], ss2[:], AF.Sqrt)
                nc.vector.tensor_scalar_max(nrm2[:], nrm2[:], 1e-12)
                inv2 = wpool.tile([P, 2], F32, tag="inv2")
                nc.vector.reciprocal(inv2[:], nrm2[:])
                smn = wpool.tile([P, 2, EMB], BF16, tag="smn")
                with nc.allow_low_precision(reason="bf16 normalized rows"):
                    nc.vector.tensor_tensor(
                        out=smn[:], in0=smf[:],
                        in1=inv2[:, :, None].to_broadcast([P, 2, EMB]),
                        op=ALU.mult,
                    )
                # ---- transpose smn -> smnT [emb-half][128, (seg-half, s)]
                smnT = []
                for eh in range(2):
                    tp = ps_tp.tile([P, 2 * P], BF16, tag="tp")
                    for half in range(2):
                        nc.tensor.transpose(
                            tp[:, half * P : (half + 1) * P],
                            smn[:, half, eh * P : (eh + 1) * P],
                            identb[:],
                        )
                    st = wpool.tile([P, 2 * P], BF16, tag=f"smnT{eh}")
                    nc.vector.tensor_copy(out=st[:], in_=tp[:])
                    smnT.append(st)
                # ---- FC + bias + batched l2norm
                xx = wpool.tile([P, 2, EMBED_SIZE], F32, tag="xx")
                for m in range(2):
                    fc_ps = ps_fc.tile([P, EMBED_SIZE], F32, tag="fc_ps")
                    for kh in range(2):
                        nc.tensor.matmul(
                            out=fc_ps[:],
                            lhsT=smnT[kh][:, m * P : (m + 1) * P],
                            rhs=fcwt_sb[kh][:],
                            start=(kh == 0),
                            stop=(kh == 1),
                            skip_group_check=True,
                        )
                    nc.vector.tensor_add(
                        out=xx[:, m, :], in0=fc_ps[:], in1=fcb_sb[:]
                    )
                sq3 = wpool.tile([P, 2, EMBED_SIZE], F32, tag="sq3")
                nc.vector.tensor_tensor(
                    out=sq3[:], in0=xx[:], in1=xx[:], op=ALU.mult
                )
                ss3 = wpool.tile([P, 2], F32, tag="ss3")
                nc.vector.tensor_reduce(
                    out=ss3[:], in_=sq3[:], axis=mybir.AxisListType.X, op=ALU.add
                )
                nrm3 = wpool.tile([P, 2], F32, tag="nrm3")
                # n = sqrt(ss + 1e-8) + 1e-8
                nc.scalar.activation(nrm3[:], ss3[:], AF.Sqrt, bias=eps8[:, 0:1])
                nc.vector.tensor_scalar_add(nrm3[:], nrm3[:], 1e-8)
                inv3 = wpool.tile([P, 2], F32, tag="inv3")
                nc.vector.reciprocal(inv3[:], nrm3[:])
                res = wpool.tile([P, 2, EMBED_SIZE], F32, tag="res")
                nc.vector.tensor_tensor(
                    out=res[:], in0=xx[:],
                    in1=inv3[:, :, None].to_broadcast([P, 2, EMBED_SIZE]),
                    op=ALU.mult,
                )
                for m in range(2):
                    nc.sync.dma_start(
                        out=out[m * P : (m + 1) * P, :], in_=res[:, m, :]
                    )
                if debug:
                    nc.sync.dma_start(out=dbg_acc[:], in_=smf[:])
                    dbg_smn_sb = wpool.tile([P, 2, EMB], F32, tag="dbgsmn")
                    nc.vector.tensor_copy(out=dbg_smn_sb[:], in_=smn[:])
                    nc.sync.dma_start(out=dbg_smn[:], in_=dbg_smn_sb[:])
                    nc.sync.dma_start(out=dbg_xx[:], in_=xx[:])

    nc.compile()
    _split_drain_waits(nc)
    return nc


# ---------------------------------------------------------------------------
# Host-side orchestration
# ---------------------------------------------------------------------------
def _phase1_prep(train_inputs, train_types, node_neigh):
    order = np.argsort(train_types, kind="stable")
    ts = train_types[order]
    tiles_s, tiles_t = [], []
    for t in range(T):
        idx_t = order[ts == t]
        if len(idx_t) == 0:
            continue
        n_tiles = -(-len(idx_t) // P)
        padded = np.concatenate(
            [idx_t, np.repeat(idx_t[-1:], n_tiles * P - len(idx_t))]
        )
        for jj in range(n_tiles):
            tiles_s.append(padded[jj * P : (jj + 1) * P])
            tiles_t.append(t)
    while len(tiles_s) % (N_CORES * NG):
        tiles_s.append(tiles_s[-1])
        tiles_t.append(tiles_t[-1])
    sample_mat = np.stack(tiles_s)  # [TT, 128]
    tile_type = np.asarray(tiles_t)
    TT = sample_mat.shape[0]
    TPC = TT // N_CORES

    flat = sample_mat.reshape(-1)
    slot_of_sample = np.zeros(B, np.int64)
    slot_of_sample[flat[::-1]] = np.arange(TT * P)[::-1]
    return sample_mat, tile_type, TPC, slot_of_sample


def _phase1_inmaps(inputs, sample_mat, tile_type, TPC):
    node_embeddings = np.asarray(inputs["node_embeddings"], np.float32)
    node_type_embeddings = np.asarray(inputs["node_type_embeddings"], np.float32)
    trans_weights = np.asarray(inputs["trans_weights"], np.float32)
    trans_weights_s1 = np.asarray(inputs["trans_weights_s1"], np.float32)
    trans_weights_s2 = np.asarray(inputs["trans_weights_s2"], np.float32)
    train_inputs = np.asarray(inputs["train_inputs"])
    node_neigh = np.asarray(inputs["node_neigh"])
    G = TPC // NG

    # ttab_r[p, c, t, u] = NTE[c*128+p, t, u]  (node-chunked, zero padded)
    nte_pad = np.zeros((NODES_PAD, T, U), np.float32)
    nte_pad[:NUM_NODES] = node_type_embeddings
    ttab_r = np.ascontiguousarray(
        nte_pad.reshape(NCHUNK, P, T, U).transpose(1, 0, 2, 3)
    ).reshape(P, NCHUNK * T * U).astype(NP_BF16)

    ntab = node_embeddings.astype(NP_BF16)

    in_maps = []
    for k in range(N_CORES):
        smp = sample_mat[k * TPC : (k + 1) * TPC]  # [TPC, 128]
        ct = tile_type[k * TPC : (k + 1) * TPC]  # [TPC]
        ne_idx = _wrap16(train_inputs[smp].reshape(-1))
        # A[p, g, c, t, s_local] neighbor count matrix
        refs = node_neigh[smp]  # [TPC, 128, 4, 10] node ids
        a_f = np.zeros((P, G, NCHUNK, T, NG * P), np.float32)
        jj = np.arange(TPC)[:, None, None, None]
        ss = np.arange(P)[None, :, None, None]
        tt = np.arange(T)[None, None, :, None]
        p_idx = (refs % P).astype(np.int64)
        c_idx = (refs // P).astype(np.int64)
        g_idx = np.broadcast_to(jj // NG, refs.shape)
        s_idx = np.broadcast_to((jj % NG) * P + ss, refs.shape)
        t_idx = np.broadcast_to(tt, refs.shape)
        np.add.at(a_f, (p_idx, g_idx, c_idx, t_idx, s_idx), 1.0)
        aarr = a_f.reshape(P, -1).astype(NP_FP8)

        s1_all = np.ascontiguousarray(
            trans_weights_s1[ct].transpose(1, 0, 2).reshape(U, TPC * DIM_A)
        ).astype(NP_BF16)
        w_all = np.ascontiguousarray(
            trans_weights[ct].transpose(1, 0, 2).reshape(U, TPC * EMB)
        ).astype(NP_BF16)
        s2_blk = np.ascontiguousarray(
            trans_weights_s2[ct][:, :, 0].T
        ).astype(NP_BF16)  # [32, TPC]
        in_maps.append(
            {
                "ttab": ttab_r,
                "aarr": aarr,
                "ntab": ntab,
                "neidx": ne_idx,
                "s1w": s1_all,
                "s2w": s2_blk,
                "ww": w_all,
            }
        )
    return in_maps


def _phase2_prep(region_index, region_segment_ids, slot_of_sample, n_rows):
    """Per-core fp8 count matrix C[p, chunk, half, seg_local]."""
    seg_ids = np.asarray(region_segment_ids).astype(np.int64)
    rows = slot_of_sample[np.asarray(region_index).astype(np.int64)]
    n_rowchunk = n_rows // P

    # dense counts [n_rows, NSEG] via bincount on linear index
    lin = rows * NSEG + seg_ids
    dense = np.bincount(lin, minlength=n_rows * NSEG).astype(np.float32)
    dense = dense.reshape(n_rowchunk, P, NSEG)  # [chunk, p, seg]

    carr_l = []
    for k in range(N_CORES):
        sl = dense[:, :, k * SEG_PER_CORE : (k + 1) * SEG_PER_CORE]
        # [p, chunk, half, seg_local]
        ck = np.ascontiguousarray(
            sl.reshape(n_rowchunk, P, 2, P).transpose(1, 0, 2, 3)
        ).reshape(P, -1).astype(NP_FP8)
        carr_l.append(ck)
    return carr_l


def _phase2_inmaps(inputs, lne_bf, carr_l):
    fc_w = np.asarray(inputs["fc_w"], np.float32)
    fc_b = np.asarray(inputs["fc_b"], np.float32)
    fcwt = np.ascontiguousarray(fc_w.T).astype(NP_BF16)  # [256, 512]
    fcb = np.broadcast_to(fc_b[None, :], (P, EMBED_SIZE)).astype(np.float32).copy()
    in_maps = []
    for k in range(N_CORES):
        in_maps.append(
            {
                "lne": lne_bf,
                "carr": carr_l[k],
                "fcwt": fcwt,
                "fcb": fcb,
            }
        )
    return in_maps


def _run_spmd_retry(nc, in_maps, retries=5, delay=30.0):
    """The axon-tunneled device occasionally reports a transient
    UNAVAILABLE/unrecoverable state; back off and retry."""
    import time as _time

    last = None
    for attempt in range(retries):
        try:
            return run_bass_kernel_spmd(nc, in_maps, list(range(N_CORES)))
        except Exception as e:  # jax.errors.JaxRuntimeError and friends
            last = e
            if attempt + 1 < retries:
                _time.sleep(delay)
    raise last


_P1_CACHE = {}
_P2_CACHE = {}


def kernel(**inputs) -> np.ndarray:
    train_inputs = np.asarray(inputs["train_inputs"])
    train_types = np.asarray(inputs["train_types"])
    node_neigh = np.asarray(inputs["node_neigh"])
    num_sms = int(inputs["num_sms"])
    max_region = int(inputs["max_region"])
    assert num_sms * max_region == NSEG

    sample_mat, tile_type, TPC, slot_of_sample = _phase1_prep(
        train_inputs, train_types, node_neigh
    )
    TT = sample_mat.shape[0]

    if TPC not in _P1_CACHE:
        _P1_CACHE[TPC] = build_phase1(TPC)
    nc1 = _P1_CACHE[TPC]
    in_maps1 = _phase1_inmaps(inputs, sample_mat, tile_type, TPC)
    res1 = _run_spmd_retry(nc1, in_maps1).results

    # relay: lne row at global_row = tile*128 + p, laid out [p, chunk, emb]
    n_rows = TT * P
    lne_bf = np.empty((P, TT, EMB), NP_BF16)
    for k in range(N_CORES):
        lne_bf[:, k * TPC : (k + 1) * TPC, :] = (
            np.asarray(res1[k]["lne"]).reshape(P, TPC, EMB)
        )
    lne_bf = lne_bf.reshape(P, TT * EMB)

    carr_l = _phase2_prep(
        inputs["region_index"], inputs["region_segment_ids"], slot_of_sample,
        n_rows,
    )
    n_rowchunk = n_rows // P
    if n_rowchunk not in _P2_CACHE:
        _P2_CACHE[n_rowchunk] = build_phase2(n_rowchunk)
    nc2 = _P2_CACHE[n_rowchunk]
    in_maps2 = _phase2_inmaps(inputs, lne_bf, carr_l)
    res2 = _run_spmd_retry(nc2, in_maps2).results

    out = np.concatenate(
        [np.asarray(res2[k]["out"], np.float32) for k in range(N_CORES)], axis=0
    )
    return out.reshape(num_sms, max_region, EMBED_SIZE)
